# revision 1
# baseline (speedup 1.0000x reference)
#!/usr/bin/env python3
"""2-layer GAT on 8 NeuronCores (Bass/Tile).

Sharding: nodes partitioned across 8 cores by dst id (graph parallel).
Per-node features computed locally, per-node gather tables allgathered,
per-edge source rows fetched with dma_gather, segment softmax/aggregation
via indicator matmuls on the tensor engine.
"""
import sys
import numpy as np

sys.path.insert(0, "/opt/pypackages")
sys.path.insert(0, "/opt/trn_rl_repo")

import concourse.bass as bass
import concourse.bacc as bacc
import concourse.tile as tile
import concourse.mybir as mybir
from concourse.bass_utils import run_bass_kernel_spmd

# problem constants
N = 100000
F_IN = 512
NHID = 16
HEADS = 8
NCLASS = 40
E = 1600000
NEG_SLOPE = 0.2

NCORES = 8
NPC = N // NCORES            # 12500 nodes per core
DCH = 128                    # dsts per chunk
NCH = (NPC + DCH - 1) // DCH  # 98 chunks
NPAD = NCH * DCH             # 12544 padded rows per core shard
NSCH = 4
SCHW = (NPAD * NCORES) // NSCH  # 25088 src rows per index window (int16-safe)

ROW1 = 256    # fp16 elems per L1 table row (512B): [h1 128 | asrc1 8 | pad]
ROW2 = 128    # fp16 elems per L2 table row (256B): [h2 40 | one | asrc2 | pad]
ROWA = 128    # fp16 elems per adst-replica row (256B)

F16 = mybir.dt.float16
F32 = mybir.dt.float32
I16 = mybir.dt.int16


def _wrap_block(v):
    """Wrap a 1-D int16 block (len % 16 == 0) into dma_gather idx layout
    [16, L/16], replicated to 128 partitions."""
    w = v.reshape(-1, 16).T
    return np.tile(w, (8, 1))



def _dma_gather_raw(gp, out_ap, in_ap, idxs_ap, num_idxs, elem_size, elem_step,
                    queue_num=0):
    """dma_gather allowing elem_size (bytes read per row) that is not a
    multiple of 256B; the table row stride (elem_step) still must be."""
    from concourse.bass import exact_div
    stride_bytes = elem_step * mybir.dt.size(in_ap.dtype)
    stride_bytes_256 = exact_div(stride_bytes, 256)
    _in_ap = gp.lower_ap_dma(in_ap, for_custom_bir_dma=True)
    _idxs_ap = gp.lower_ap(idxs_ap)
    _out_ap = gp.lower_ap(out_ap)
    return gp.add_instruction(
        mybir.InstDMAGatherAnt(
            name=gp.bass.get_next_instruction_name(),
            ins=[*_in_ap, _idxs_ap, gp.lower_val_access(gp.to_reg(num_idxs))],
            outs=[_out_ap],
            transpose=False, num_idxs=num_idxs, elem_size=elem_size,
            stride_bytes_256=stride_bytes_256, gen_mode=0,
            single_packet=False, queue_num=queue_num,
            sbuf_tokens_per_rank=0, sbuf_free_dim_per_rank=0,
            sbuf_free_dim_pad_per_rank=0, sbuf_byte_offset=0))


def _prep(x, edge_index, W1, att_src1, att_dst1, W2, att_src2, att_dst2,
          b1=None, b2=None):
    """Host-side sharding/packing. Returns (in_maps, shapes) where shapes is
    the static cell structure shared by all cores."""
    src = np.concatenate([edge_index[0], np.arange(N, dtype=np.int64)])
    dst = np.concatenate([edge_index[1], np.arange(N, dtype=np.int64)])

    core = dst // NPC
    dl = (dst - core * NPC).astype(np.int64)      # local dst 0..12499
    dch = dl >> 7                                  # dst chunk 0..97
    s_pad = (src // NPC) * NPAD + (src % NPC)      # padded global src row
    sch = s_pad // SCHW
    sloc = (s_pad - sch * SCHW).astype(np.int64)   # 0..25087 (int16 ok)

    cell = ((core * NCH + dch) * NSCH + sch).astype(np.int64)
    order = np.argsort(cell * (SCHW + 1) + sloc, kind="stable")
    cell_s, sloc_s, dl_s = cell[order], sloc[order], dl[order]

    ncells = NCORES * NCH * NSCH
    counts = np.bincount(cell_s, minlength=ncells).reshape(NCORES, NCH * NSCH)
    shapes = (np.ceil(counts.max(axis=0) / 128.0).astype(np.int64) * 128)  # [NCH*NSCH]
    cell_starts = np.concatenate([[0], np.cumsum(shapes)])                 # per-core stream offsets
    t_total = int(cell_starts[-1]) // 128

    # rank of each edge within its cell
    group_start = np.concatenate([[0], np.cumsum(counts.reshape(-1))])
    first_of_cell = group_start[cell_s]
    rank = np.arange(len(cell_s)) - first_of_cell
    # destination position within the owning core's padded stream
    pos = cell_starts[cell_s % (NCH * NSCH)] + rank
    core_s = cell_s // (NCH * NSCH)

    L = t_total * 128
    idx1 = np.zeros((NCORES, L), dtype=np.int16)
    idxd = np.zeros((NCORES, L), dtype=np.int16)
    dstloc = np.full((NCORES, L), 255.0, dtype=np.float16)
    idx1[core_s, pos] = sloc_s.astype(np.int16)
    idxd[core_s, pos] = dl_s.astype(np.int16)
    dstloc[core_s, pos] = (dl_s & 127).astype(np.float16)

    # per-chunk tile counts and cell layout
    shapes2 = shapes.reshape(NCH, NSCH)
    # wrapped idx streams
    IDX1 = np.zeros((NCORES, 128, L // 16), dtype=np.int16)
    IDXD = np.zeros((NCORES, 128, L // 16), dtype=np.int16)
    for k in range(NCORES):
        off = 0
        for d in range(NCH):
            chunk_len = int(shapes2[d].sum())
            if chunk_len:
                blk = idxd[k, off:off + chunk_len]
                IDXD[k][:, off // 16:(off + chunk_len) // 16] = _wrap_block(blk)
            coff = off
            for s in range(NSCH):
                cl = int(shapes2[d, s])
                if cl:
                    blk = idx1[k, coff:coff + cl]
                    IDX1[k][:, coff // 16:(coff + cl) // 16] = _wrap_block(blk)
                coff += cl
            off += chunk_len
    DSTLOC = dstloc.reshape(NCORES, t_total, 128).transpose(0, 2, 1).copy()

    # weights
    asrc1 = att_src1.reshape(HEADS, NHID)
    adst1 = att_dst1.reshape(HEADS, NHID)
    W1r = W1.reshape(F_IN, HEADS, NHID)
    W1as = np.einsum("khc,hc->kh", W1r, asrc1)     # [512, 8]
    W1ad = np.einsum("khc,hc->kh", W1r, adst1)
    W1ext = np.concatenate([W1, W1as, W1ad], axis=1).astype(np.float16)  # [512, 144]
    W2as = W2 @ att_src2.reshape(NCLASS, 1)        # [128, 1]
    W2ad = W2 @ att_dst2.reshape(NCLASS, 1)
    W2ext = np.concatenate([W2, W2as, W2ad], axis=1).astype(np.float16)  # [128, 42]

    iota = np.broadcast_to(np.arange(128, dtype=np.float16), (128, 128)).copy()

    in_maps = []
    for k in range(NCORES):
        xs = x[k * NPC:(k + 1) * NPC]              # [12500, 512]
        xT = np.zeros((F_IN, NPAD), dtype=np.float16)
        xT[:, :NPC] = xs.T
        in_maps.append({
            "xT": xT,
            "W1ext": W1ext,
            "W2ext": W2ext,
            "IDX1": IDX1[k],
            "IDXD": IDXD[k],
            "DSTLOC": DSTLOC[k],
            "iota": iota,
            "B1": (np.zeros((1, 128), np.float32) if b1 is None
                   else np.asarray(b1, np.float32).reshape(1, 128)),
            "B2": (np.zeros((1, NCLASS), np.float32) if b2 is None
                   else np.asarray(b2, np.float32).reshape(1, NCLASS)),
        })
    return in_maps, shapes2


def _build(shapes2, nch=NCH, phases="ABCDE", clevel=9):
    """Build the Bass module given the static cell structure [NCH, NSCH]."""
    from concourse.masks import make_identity

    t_chunks = [int(shapes2[d].sum()) // 128 for d in range(NCH)]
    t_total = sum(t_chunks)
    TMAX = max(t_chunks)

    nc = bacc.Bacc("TRN2", target_bir_lowering=False, debug=False,
                   enable_asserts=False, num_devices=NCORES,
                   num_swdge_queues=4)

    xT = nc.dram_tensor("xT", [F_IN, NPAD], F16, kind="ExternalInput")
    W1e = nc.dram_tensor("W1ext", [F_IN, 144], F16, kind="ExternalInput")
    W2e = nc.dram_tensor("W2ext", [128, 42], F16, kind="ExternalInput")
    IDX1 = nc.dram_tensor("IDX1", [128, t_total * 8], I16, kind="ExternalInput")
    IDXD = nc.dram_tensor("IDXD", [128, t_total * 8], I16, kind="ExternalInput")
    DSTLOC = nc.dram_tensor("DSTLOC", [128, t_total], F16, kind="ExternalInput")
    IOTA = nc.dram_tensor("iota", [128, 128], F16, kind="ExternalInput")
    B1 = nc.dram_tensor("B1", [1, 128], F32, kind="ExternalInput")
    B2 = nc.dram_tensor("B2", [1, NCLASS], F32, kind="ExternalInput")
    OUT = nc.dram_tensor("out", [NPAD, NCLASS], F32, kind="ExternalOutput")

    tab1_sh = nc.dram_tensor("tab1_sh", [NPAD, ROW1], F16, kind="Internal")
    tab1 = nc.dram_tensor("tab1", [NPAD * NCORES, ROW1], F16, kind="Internal",
                          addr_space="Shared")
    tab2_sh = nc.dram_tensor("tab2_sh", [NPAD, ROW2], F16, kind="Internal")
    tab2 = nc.dram_tensor("tab2", [NPAD * NCORES, ROW2], F16, kind="Internal",
                          addr_space="Shared")
    adr1 = nc.dram_tensor("adr1", [NPAD, ROWA], F16, kind="Internal")
    adr2 = nc.dram_tensor("adr2", [NPAD, ROWA], F16, kind="Internal")

    eq = mybir.AluOpType.is_equal
    mult = mybir.AluOpType.mult
    amax = mybir.AluOpType.max
    aadd = mybir.AluOpType.add
    sub = mybir.AluOpType.subtract
    AF = mybir.ActivationFunctionType
    AX = mybir.AxisListType

    with tile.TileContext(nc) as tc:
        if "A" in phases:
            _phase_a(nc, tc, nch, xT, W1e, tab1_sh, adr1)
        if "B" in phases:
            nc.gpsimd.collective_compute(
                "AllGather", mybir.AluOpType.bypass,
                replica_groups=[list(range(NCORES))],
                ins=[tab1_sh[:]], outs=[tab1[:]])
        if "C" in phases:
            _phase_c(nc, tc, nch, shapes2, t_chunks, TMAX, make_identity,
                     IDX1, IDXD, DSTLOC, IOTA, B1, W2e, tab1, adr1, tab2_sh, adr2,
                     eq, mult, amax, aadd, AF, clevel)
        if "D" in phases:
            nc.gpsimd.collective_compute(
                "AllGather", mybir.AluOpType.bypass,
                replica_groups=[list(range(NCORES))],
                ins=[tab2_sh[:]], outs=[tab2[:]])
        if "E" in phases:
            _phase_e(nc, tc, nch, shapes2, t_chunks, TMAX,
                     IDX1, IDXD, DSTLOC, IOTA, B2, tab2, adr2, OUT,
                     eq, mult, amax, aadd, sub, AF, AX)

    nc.compile()
    return nc


def _phase_a(nc, tc, nch, xT, W1e, tab1_sh, adr1):
    with tc.tile_pool(name="sbA", bufs=1) as sba, \
         tc.tile_pool(name="sbA2", bufs=4) as sba2, \
         tc.tile_pool(name="psA", bufs=4, space="PSUM") as psa:
        xts = [sba.tile([128, NPAD], F16, tag=f"xt{k}", name=f"xt{k}")
               for k in range(4)]
        w1s = [sba.tile([128, 144], F16, tag=f"w1{k}", name=f"w1{k}")
               for k in range(4)]
        for k in range(4):
            nc.sync.dma_start(xts[k][:], xT[k * 128:(k + 1) * 128, :])
            nc.sync.dma_start(w1s[k][:], W1e[k * 128:(k + 1) * 128, :])
        for nt in range(nch):
            ps = psa.tile([128, 144], F32, tag="psA", name="psA")
            for k in range(4):
                nc.tensor.matmul(ps[:], lhsT=xts[k][:, nt * 128:(nt + 1) * 128],
                                 rhs=w1s[k][:], start=(k == 0), stop=(k == 3))
            row = sba2.tile([128, 136], F16, tag="row", name="row")
            nc.vector.tensor_copy(row[:], ps[:, 0:136])
            nc.sync.dma_start(tab1_sh[nt * 128:(nt + 1) * 128, 0:136], row[:])
            t8 = sba2.tile([128, 8], F16, tag="t8", name="t8")
            nc.vector.tensor_copy(t8[:], ps[:, 136:144])
            nc.sync.dma_start(adr1[nt * 128:(nt + 1) * 128, 0:8], t8[:])


def _phase_c(nc, tc, nch, shapes2, t_chunks, TMAX, make_identity,
             IDX1, IDXD, DSTLOC, IOTA, B1, W2e, tab1, adr1, tab2_sh, adr2,
             eq, mult, amax, aadd, AF, clevel=9):
    with tc.tile_pool(name="sbC", bufs=1) as sbc, \
         tc.tile_pool(name="sbC2", bufs=3) as sb2, \
         tc.tile_pool(name="psC", bufs=2, space="PSUM") as psc:
        iot = sbc.tile([128, 128], F16, tag="iota", name="iotc")
        nc.sync.dma_start(iot[:], IOTA[:])
        ident = sbc.tile([128, 128], F16, tag="ident", name="ident")
        make_identity(nc, ident[:])
        w2s = sbc.tile([128, 42], F16, tag="w2s", name="w2s")
        nc.sync.dma_start(w2s[:], W2e[:])
        b1t = sbc.tile([128, 128], F32, tag="b1t", name="b1t")
        nc.sync.dma_start(b1t[:], B1[:].to_broadcast([128, 128]))

        off = 0  # tile offset into the edge stream
        for d in range(nch):
            T = t_chunks[d]
            if T == 0:
                continue
            i1 = sb2.tile([128, TMAX * 8], I16, tag="i1", name="i1")
            nc.sync.dma_start(i1[:, 0:T * 8], IDX1[:, off * 8:(off + T) * 8])
            idd = sb2.tile([128, TMAX * 8], I16, tag="idd", name="idd")
            nc.sync.dma_start(idd[:, 0:T * 8], IDXD[:, off * 8:(off + T) * 8])
            dlc = sb2.tile([128, TMAX], F16, tag="dlc", name="dlc")
            nc.sync.dma_start(dlc[:, 0:T], DSTLOC[:, off:off + T])

            g1 = sb2.tile([128, TMAX * ROW1], F16, tag="g1", name="g1")
            coff = 0
            for s in range(NSCH):
                cl = int(shapes2[d, s])
                if cl == 0:
                    continue
                if clevel >= 1:
                    nc.gpsimd.dma_gather(
                        out_ap=g1[:, coff * 2:(coff * 2 + (cl // 128) * ROW1)]
                        .rearrange("p (t e) -> p t e", e=ROW1),
                        in_ap=tab1[s * SCHW:(s + 1) * SCHW, :],
                        idxs_ap=i1[:, coff // 16:(coff + cl) // 16],
                        num_idxs=cl, num_idxs_reg=cl, elem_size=ROW1, single_packet=False)
                coff += cl
            ga = sb2.tile([128, TMAX * 8], F16, tag="ga", name="ga")
            nedge = T * 128
            _dma_gather_raw(nc.gpsimd,
                            ga[:, 0:T * 8].rearrange("p (t e) -> p t e", e=8),
                            adr1[:], idd[:, 0:nedge // 16], nedge, 8, ROWA,
                            queue_num=d % 4)

            if clevel < 2:
                dbg = sb2.tile([128, 128], F16, tag="dbg", name="dbg")
                nc.vector.tensor_copy(dbg[:], ga[:, 0:128] if clevel < 1 else g1[:, 0:128])
                nc.sync.dma_start(tab2_sh[d * 128:(d + 1) * 128, 0:128], dbg[:])
                off += T
                continue
            g13 = g1[:, 0:T * ROW1].rearrange("p (t e) -> p t e", e=ROW1)
            ga3 = ga[:, 0:T * 8].rearrange("p (t e) -> p t e", e=8)

            ind = sb2.tile([128, TMAX * 128], F16, tag="ind", name="ind")
            ind3 = ind[:, 0:T * 128].rearrange("p (t s) -> p t s", s=128)
            nc.vector.tensor_tensor(
                out=ind3,
                in0=iot[:].rearrange("p (t s) -> p t s", t=1)
                .to_broadcast([128, T, 128]),
                in1=dlc[:, 0:T].rearrange("p (t s) -> p t s", s=1)
                .to_broadcast([128, T, 128]),
                op=eq)

            att = sb2.tile([128, TMAX * 8], F16, tag="att", name="att")
            at3 = att[:, 0:T * 8].rearrange("p (t h) -> p t h", h=8)
            nc.vector.tensor_tensor(out=at3, in0=g13[:, :, 128:136],
                                    in1=ga3[:, :, 0:8], op=aadd)
            nc.vector.scalar_tensor_tensor(
                out=at3, in0=at3, scalar=NEG_SLOPE, in1=at3, op0=mult, op1=amax)
            wst = sb2.tile([128, TMAX * 8], F16, tag="wst", name="wst")
            nc.scalar.activation(out=wst[:, 0:T * 8], in_=att[:, 0:T * 8],
                                 func=AF.Exp)

            if clevel < 3:
                dbg = sb2.tile([128, 128], F16, tag="dbg", name="dbg")
                nc.vector.tensor_copy(dbg[:, 0:120], ind[:, 0:120])
                nc.vector.tensor_copy(dbg[:, 120:128], wst[:, 0:8])
                nc.sync.dma_start(tab2_sh[d * 128:(d + 1) * 128, 0:128], dbg[:])
                off += T
                continue
            ust = sb2.tile([128, TMAX * 136], F16, tag="ust", name="ust")
            us3 = ust[:, 0:T * 136].rearrange("p (t e) -> p t e", e=136)
            w3 = wst[:, 0:T * 8].rearrange("p (t h) -> p t h", h=8)
            nc.vector.tensor_tensor(
                out=ust[:, 0:T * 136].rearrange("p (t e) -> p t e", e=136)[:, :, 0:128]
                .rearrange("p t (h c) -> p t h c", c=NHID),
                in0=g1[:, 0:T * ROW1].rearrange("p (t e) -> p t e", e=ROW1)[:, :, 0:128]
                .rearrange("p t (h c) -> p t h c", c=NHID),
                in1=wst[:, 0:T * 8].rearrange("p (t h c) -> p t h c", h=8, c=1)
                .to_broadcast([128, T, 8, NHID]),
                op=mult)
            nc.vector.tensor_copy(us3[:, :, 128:136], w3)

            ps1 = psc.tile([128, 136], F32, tag="ps1", name="ps1")
            for t in range(T):
                nc.tensor.matmul(ps1[:], lhsT=ind[:, t * 128:(t + 1) * 128],
                                 rhs=ust[:, t * 136:(t + 1) * 136],
                                 start=(t == 0), stop=(t == T - 1))

            if clevel < 4:
                dbg = sb2.tile([128, 128], F16, tag="dbg", name="dbg")
                nc.vector.tensor_copy(dbg[:], ps1[:, 0:128])
                nc.sync.dma_start(tab2_sh[d * 128:(d + 1) * 128, 0:128], dbg[:])
                off += T
                continue
            rc = sb2.tile([128, 8], F32, tag="rc", name="rc")
            nc.vector.reciprocal(rc[:], ps1[:, 128:136])
            o1 = sb2.tile([128, 128], F32, tag="o1", name="o1")
            nc.vector.tensor_tensor(
                out=o1[:].rearrange("p (h c) -> p h c", c=NHID),
                in0=ps1[:, 0:128].rearrange("p (h c) -> p h c", c=NHID),
                in1=rc[:].rearrange("p (h c) -> p h c", c=1)
                .to_broadcast([128, 8, NHID]),
                op=mult)
            nc.vector.tensor_tensor(out=o1[:], in0=o1[:], in1=b1t[:], op=aadd)
            # elu = max(x,0) + (exp(min(x,0)) - 1)
            t1 = sb2.tile([128, 128], F32, tag="t1", name="t1")
            nc.vector.tensor_scalar_min(t1[:], o1[:], 0.0)
            t2 = sb2.tile([128, 128], F32, tag="t2", name="t2")
            nc.scalar.activation(out=t2[:], in_=t1[:], func=AF.Exp)
            nc.vector.tensor_scalar_add(t2[:], t2[:], -1.0)
            nc.vector.tensor_scalar_max(t1[:], o1[:], 0.0)
            elu = sb2.tile([128, 128], F16, tag="elu", name="elu")
            nc.vector.tensor_tensor(out=elu[:], in0=t1[:], in1=t2[:], op=aadd)

            if clevel < 5:
                nc.sync.dma_start(tab2_sh[d * 128:(d + 1) * 128, 0:128], elu[:])
                off += T
                continue
            psT = psc.tile([128, 128], F16, tag="psT", name="psT")
            nc.tensor.transpose(psT[:], elu[:], ident[:])
            eluT = sb2.tile([128, 128], F16, tag="eluT", name="eluT")
            nc.vector.tensor_copy(eluT[:], psT[:])
            ps2a = psc.tile([128, 42], F32, tag="ps2a", name="ps2a")
            nc.tensor.matmul(ps2a[:], lhsT=eluT[:], rhs=w2s[:],
                             start=True, stop=True)

            h2r = sb2.tile([128, ROW2], F16, tag="h2r", name="h2r")
            nc.vector.tensor_copy(h2r[:, 0:NCLASS], ps2a[:, 0:NCLASS])
            nc.vector.memset(h2r[:, NCLASS:NCLASS + 1], 1.0)
            nc.vector.tensor_copy(h2r[:, NCLASS + 1:NCLASS + 2],
                                  ps2a[:, NCLASS:NCLASS + 1])
            nc.sync.dma_start(tab2_sh[d * 128:(d + 1) * 128, 0:NCLASS + 2],
                              h2r[:, 0:NCLASS + 2])
            a2c = sb2.tile([128, 8], F16, tag="a2c", name="a2c")
            nc.vector.tensor_copy(
                a2c[:].rearrange("p (r h) -> p r h", h=1),
                ps2a[:, 41:42].rearrange("p (r h) -> p r h", r=1)
                .to_broadcast([128, 8, 1]))
            nc.sync.dma_start(adr2[d * 128:(d + 1) * 128, 0:8], a2c[:])
            off += T


def _phase_e(nc, tc, nch, shapes2, t_chunks, TMAX,
             IDX1, IDXD, DSTLOC, IOTA, B2, tab2, adr2, OUT,
             eq, mult, amax, aadd, sub, AF, AX):
    with tc.tile_pool(name="sbE", bufs=1) as sbe, \
         tc.tile_pool(name="sbE2", bufs=3) as se2, \
         tc.tile_pool(name="psE", bufs=4, space="PSUM") as pse:
        iot = sbe.tile([128, 128], F16, tag="iotaE", name="iote")
        nc.sync.dma_start(iot[:], IOTA[:])
        b2t = sbe.tile([128, NCLASS], F32, tag="b2t", name="b2t")
        nc.sync.dma_start(b2t[:], B2[:].to_broadcast([128, NCLASS]))
        off = 0
        for d in range(nch):
            T = t_chunks[d]
            if T == 0:
                continue
            i1 = se2.tile([128, TMAX * 8], I16, tag="i1e", name="i1e")
            nc.sync.dma_start(i1[:, 0:T * 8], IDX1[:, off * 8:(off + T) * 8])
            idd = se2.tile([128, TMAX * 8], I16, tag="idde", name="idde")
            nc.sync.dma_start(idd[:, 0:T * 8], IDXD[:, off * 8:(off + T) * 8])
            dlc = se2.tile([128, TMAX], F16, tag="dlce", name="dlce")
            nc.sync.dma_start(dlc[:, 0:T], DSTLOC[:, off:off + T])

            g2 = se2.tile([128, TMAX * 42], F16, tag="g2", name="g2")
            coff = 0
            for s in range(NSCH):
                cl = int(shapes2[d, s])
                if cl == 0:
                    continue
                _dma_gather_raw(nc.gpsimd,
                                g2[:, (coff // 128) * 42:((coff + cl) // 128) * 42]
                                .rearrange("p (t e) -> p t e", e=42),
                                tab2[s * SCHW:(s + 1) * SCHW, :],
                                i1[:, coff // 16:(coff + cl) // 16], cl, 42, ROW2,
                                queue_num=s)
                coff += cl
            ga2 = se2.tile([128, TMAX * 8], F16, tag="ga2", name="ga2")
            nedge = T * 128
            _dma_gather_raw(nc.gpsimd,
                            ga2[:, 0:T * 8].rearrange("p (t e) -> p t e", e=8),
                            adr2[:], idd[:, 0:nedge // 16], nedge, 8, ROWA,
                            queue_num=d % 4)

            g23 = g2[:, 0:T * 42].rearrange("p (t e) -> p t e", e=42)
            ga23 = ga2[:, 0:T * 8].rearrange("p (t e) -> p t e", e=8)

            ind = se2.tile([128, TMAX * 128], F16, tag="inde", name="inde")
            ind3 = ind[:, 0:T * 128].rearrange("p (t s) -> p t s", s=128)
            nc.vector.tensor_tensor(
                out=ind3,
                in0=iot[:].rearrange("p (t s) -> p t s", t=1)
                .to_broadcast([128, T, 128]),
                in1=dlc[:, 0:T].rearrange("p (t s) -> p t s", s=1)
                .to_broadcast([128, T, 128]),
                op=eq)

            at2 = se2.tile([128, TMAX], F16, tag="at2", name="at2")
            at23 = at2[:, 0:T].rearrange("p (t h) -> p t h", h=1)
            nc.vector.tensor_tensor(out=at23,
                                    in0=g23[:, :, NCLASS + 1:NCLASS + 2],
                                    in1=ga23[:, :, 0:1], op=aadd)
            nc.vector.scalar_tensor_tensor(
                out=at23, in0=at23, scalar=NEG_SLOPE, in1=at23,
                op0=mult, op1=amax)
            w2t = se2.tile([128, TMAX], F16, tag="w2t", name="w2t")
            nc.scalar.activation(out=w2t[:, 0:T], in_=at2[:, 0:T], func=AF.Exp)

            gw = se2.tile([128, TMAX * 42], F16, tag="gw", name="gw")
            nc.vector.tensor_tensor(
                out=gw[:, 0:T * 42].rearrange("p (t e) -> p t e", e=42),
                in0=g23,
                in1=w2t[:, 0:T].rearrange("p (t s) -> p t s", s=1)
                .to_broadcast([128, T, 42]),
                op=mult)

            ps2 = pse.tile([128, NCLASS + 1], F32, tag="ps2", name="ps2")
            for t in range(T):
                nc.tensor.matmul(ps2[:], lhsT=ind[:, t * 128:(t + 1) * 128],
                                 rhs=gw[:, t * 42:t * 42 + NCLASS + 1],
                                 start=(t == 0), stop=(t == T - 1))

            rc2 = se2.tile([128, 1], F32, tag="rc2", name="rc2")
            nc.vector.reciprocal(rc2[:], ps2[:, NCLASS:NCLASS + 1])
            lg = se2.tile([128, NCLASS], F32, tag="lg", name="lg")
            nc.vector.scalar_tensor_tensor(out=lg[:], in0=ps2[:, 0:NCLASS],
                                           scalar=rc2[:], in1=b2t[:],
                                           op0=mult, op1=aadd)
            ex = se2.tile([128, NCLASS], F32, tag="ex", name="ex")
            sm = se2.tile([128, 1], F32, tag="sm", name="sm")
            nc.scalar.activation(out=ex[:], in_=lg[:], func=AF.Exp,
                                 accum_out=sm[:])
            ls = se2.tile([128, 1], F32, tag="ls", name="ls")
            nc.scalar.activation(out=ls[:], in_=sm[:], func=AF.Ln)
            fin = se2.tile([128, NCLASS], F32, tag="fin", name="fin")
            nc.vector.tensor_scalar(out=fin[:], in0=lg[:], scalar1=ls[:],
                                    scalar2=None, op0=sub)
            nc.sync.dma_start(OUT[d * 128:(d + 1) * 128, :], fin[:])
            off += T


_CACHE = {}


def kernel(x, edge_index, W1, att_src1, att_dst1, b1, W2, att_src2, att_dst2, b2):
    x = np.asarray(x, dtype=np.float32)
    edge_index = np.asarray(edge_index)
    in_maps, shapes2 = _prep(np.asarray(x), edge_index,
                             np.asarray(W1), np.asarray(att_src1),
                             np.asarray(att_dst1), np.asarray(W2),
                             np.asarray(att_src2), np.asarray(att_dst2),
                             b1=b1, b2=b2)
    key = shapes2.tobytes()
    if key not in _CACHE:
        _CACHE[key] = _build(shapes2)
    nc = _CACHE[key]
    res = run_bass_kernel_spmd(nc, in_maps, core_ids=list(range(NCORES)))
    out = np.concatenate([res.results[k]["out"][:NPC] for k in range(NCORES)], axis=0)
    return out.astype(np.float32)



# revision 2
# speedup vs baseline: 2.1167x; 2.1167x over previous
#!/usr/bin/env python3
"""2-layer GAT on 8 NeuronCores (Bass/Tile) — v3.

v2 + tail-only padding: edges are packed per (group, window) segment
(sorted by chunk within the segment) and padded only at the segment tail,
instead of 128-padding every (chunk, window) cell. Descriptor streams are
trimmed to the cross-core max valid count per segment (dummy idx-0 rows up
to the max keep the SPMD descriptor count identical on every core, then a
trailing run of -1 indices that the DGE skips). Tiles may straddle chunk
boundaries; the indicator is built per (chunk, window) tile range with an
absolute int16 dst-id compare, so foreign edges in a shared tile hit no
indicator column. Cuts ~25% of gather descriptors, DVE elements, and PE
matmul tiles.
"""
import sys
import numpy as np

sys.path.insert(0, "/opt/pypackages")
sys.path.insert(0, "/opt/trn_rl_repo")

import concourse.bass as bass
import concourse.bacc as bacc
import concourse.tile as tile
import concourse.mybir as mybir
from concourse.bass_utils import run_bass_kernel_spmd

# problem constants
N = 100000
F_IN = 512
NHID = 16
HEADS = 8
NCLASS = 40
E = 1600000
NEG_SLOPE = 0.2

NCORES = 8
NPC = N // NCORES
DCH = 128
NCH = (NPC + DCH - 1) // DCH
NPAD = NCH * DCH             # 12544
NSCH = 4
SCHW = (NPAD * NCORES) // NSCH  # 25088
G1 = 3
NG = (NCH + G1 - 1) // G1

ROW1 = 256
U1 = 136
ROW2 = 128
U2 = 42
ROWA = 128

F16 = mybir.dt.float16
F32 = mybir.dt.float32
I16 = mybir.dt.int16


def _wrap_block(v):
    w = v.reshape(-1, 16).T
    return np.tile(w, (8, 1))


def _dma_gather_raw(gp, out_ap, in_ap, idxs_ap, num_idxs, num_valid,
                    elem_size, elem_step, queue_num=0):
    """dma_gather with elem_size not a multiple of 256B and explicit valid
    count (trailing -1 indices are skipped by the DGE)."""
    from concourse.bass import exact_div
    stride_bytes = elem_step * mybir.dt.size(in_ap.dtype)
    stride_bytes_256 = exact_div(stride_bytes, 256)
    _in_ap = gp.lower_ap_dma(in_ap, for_custom_bir_dma=True)
    _idxs_ap = gp.lower_ap(idxs_ap)
    _out_ap = gp.lower_ap(out_ap)
    return gp.add_instruction(
        mybir.InstDMAGatherAnt(
            name=gp.bass.get_next_instruction_name(),
            ins=[*_in_ap, _idxs_ap, gp.lower_val_access(gp.to_reg(num_valid))],
            outs=[_out_ap],
            transpose=False, num_idxs=num_idxs, elem_size=elem_size,
            stride_bytes_256=stride_bytes_256, gen_mode=0,
            single_packet=False, queue_num=queue_num,
            sbuf_tokens_per_rank=0, sbuf_free_dim_per_rank=0,
            sbuf_free_dim_pad_per_rank=0, sbuf_byte_offset=0))


def _prep(x, edge_index, W1, att_src1, att_dst1, W2, att_src2, att_dst2,
          b1=None, b2=None):
    """Returns (in_maps, meta) where meta carries the static structure
    shared by all cores: seg_shapes [NG,NSCH] (128-rounded segment lens),
    seg_maxcnt [NG,NSCH] (cross-core max valid count), and ranges
    [NCH,NSCH,2] (tile [a,b) of each chunk within its segment)."""
    # self-loop contributions are computed node-locally on device (no
    # gather), so the edge stream holds only the real edges
    src = np.asarray(edge_index[0], dtype=np.int64)
    dst = np.asarray(edge_index[1], dtype=np.int64)

    core = dst // NPC
    dl = (dst - core * NPC).astype(np.int64)
    dch = dl >> 7
    gg = dch // G1
    c4 = dch % G1
    s_pad = (src // NPC) * NPAD + (src % NPC)
    sch = s_pad // SCHW
    sloc = (s_pad - sch * SCHW).astype(np.int64)

    seg = (core * NG + gg) * NSCH + sch             # (core, g, s) segment id
    sub = c4 * (SCHW + 1) + sloc                    # order by chunk then src
    order = np.argsort(seg * (G1 * (SCHW + 1) + 2) + sub, kind="stable")
    seg_s, sloc_s, dl_s = seg[order], sloc[order], dl[order]
    c4_s = c4[order]

    nseg = NG * NSCH
    counts = np.bincount(seg_s, minlength=NCORES * nseg).reshape(NCORES, nseg)
    seg_maxcnt = counts.max(axis=0)                          # [nseg]
    seg_shapes = (np.ceil(seg_maxcnt / 128.0).astype(np.int64) * 128)
    seg_starts = np.concatenate([[0], np.cumsum(seg_shapes)])
    t_total = int(seg_starts[-1]) // 128

    # per-(core, g, s, c4) counts for the static chunk tile ranges
    cell4 = seg_s * G1 + c4_s
    counts4 = np.bincount(cell4, minlength=NCORES * nseg * G1) \
        .reshape(NCORES, nseg, G1)
    pre4 = np.concatenate([np.zeros((NCORES, nseg, 1), np.int64),
                           np.cumsum(counts4, axis=2)], axis=2)
    a4 = pre4[:, :, :G1].min(axis=0) // 128                  # [nseg, G1]
    b4 = -(-pre4[:, :, 1:].max(axis=0) // 128)               # ceil
    b4 = np.minimum(b4, (seg_shapes // 128)[:, None])
    a4 = np.minimum(a4, b4)

    seg_first = np.concatenate([[0], np.cumsum(counts.reshape(-1))])
    rank = np.arange(len(seg_s)) - seg_first[seg_s]
    pos = seg_starts[seg_s % nseg] + rank
    core_s = seg_s // nseg

    L = t_total * 128
    # default -1 (skipped); dummy 0 up to maxcnt set below
    idx1 = np.full((NCORES, L), -1, dtype=np.int16)
    idxd = np.full((NCORES, L), -1, dtype=np.int16)
    dsti = np.full((NCORES, L), -1, dtype=np.int16)
    idx1[core_s, pos] = sloc_s.astype(np.int16)
    idxd[core_s, pos] = dl_s.astype(np.int16)
    dsti[core_s, pos] = dl_s.astype(np.int16)
    # dummy idx-0 rows in [count_core, maxcnt) keep the descriptor count
    # static across cores (dsti stays -1 so they hit no indicator column)
    for k in range(NCORES):
        for sg in range(nseg):
            lo = seg_starts[sg] + counts[k, sg]
            hi = seg_starts[sg] + seg_maxcnt[sg]
            idx1[k, lo:hi] = 0
            idxd[k, lo:hi] = 0

    shapes2 = seg_shapes.reshape(NG, NSCH)
    maxcnt2 = seg_maxcnt.reshape(NG, NSCH)
    ranges = np.stack([a4, b4], axis=2).reshape(NG, NSCH, G1, 2)

    IDX1 = np.zeros((NCORES, 128, L // 16), dtype=np.int16)
    IDXD = np.zeros((NCORES, 128, L // 16), dtype=np.int16)
    for k in range(NCORES):
        off = 0
        for g in range(NG):
            for s in range(NSCH):
                cl = int(shapes2[g, s])
                if cl:
                    IDX1[k][:, off // 16:(off + cl) // 16] = \
                        _wrap_block(idx1[k, off:off + cl])
                    IDXD[k][:, off // 16:(off + cl) // 16] = \
                        _wrap_block(idxd[k, off:off + cl])
                off += cl
    DSTI = dsti.reshape(NCORES, t_total, 128).transpose(0, 2, 1).copy()

    asrc1 = att_src1.reshape(HEADS, NHID)
    adst1 = att_dst1.reshape(HEADS, NHID)
    W1r = W1.reshape(F_IN, HEADS, NHID)
    W1as = np.einsum("khc,hc->kh", W1r, asrc1)
    W1ad = np.einsum("khc,hc->kh", W1r, adst1)
    W1ext = np.concatenate([W1, W1as, W1ad], axis=1).astype(np.float16)
    W2as = W2 @ att_src2.reshape(NCLASS, 1)
    W2ad = W2 @ att_dst2.reshape(NCLASS, 1)
    W2ext = np.concatenate([W2, np.zeros((HEADS * NHID, 1)), W2as, W2ad],
                           axis=1).astype(np.float16)

    iotaa = np.broadcast_to(np.arange(NPAD, dtype=np.int16), (128, NPAD)).copy()

    in_maps = []
    for k in range(NCORES):
        xs = x[k * NPC:(k + 1) * NPC]
        xT = np.zeros((F_IN, NPAD), dtype=np.float16)
        xT[:, :NPC] = xs.T
        in_maps.append({
            "xT": xT,
            "W1ext": W1ext,
            "W2ext": W2ext,
            "IDX1": IDX1[k],
            "IDXD": IDXD[k],
            "DSTI": DSTI[k],
            "iotaa": iotaa,
            "B1": (np.zeros((1, 128), np.float16) if b1 is None
                   else np.asarray(b1, np.float16).reshape(1, 128)),
            "B2": (np.zeros((1, NCLASS), np.float16) if b2 is None
                   else np.asarray(b2, np.float16).reshape(1, NCLASS)),
        })
    return in_maps, (shapes2, maxcnt2, ranges)


def _build(meta, phases="ABCDE"):
    from concourse.masks import make_identity
    shapes2, maxcnt2, ranges = meta
    shapes2 = np.asarray(shapes2)
    maxcnt2 = np.asarray(maxcnt2)
    ranges = np.asarray(ranges)
    ng = shapes2.shape[0]
    g_tiles = [int(shapes2[g].sum()) // 128 for g in range(ng)]
    t_total = sum(g_tiles)
    TGMAX = max(g_tiles)
    g_off = np.concatenate([[0], np.cumsum(g_tiles)]).astype(int)
    # max per-chunk indicator tiles
    TCMAX = int((ranges[:, :, :, 1] - ranges[:, :, :, 0]).sum(axis=1).max())

    nc = bacc.Bacc("TRN2", target_bir_lowering=False, debug=False,
                   enable_asserts=False, num_devices=NCORES,
                   num_swdge_queues=4)

    xT = nc.dram_tensor("xT", [F_IN, NPAD], F16, kind="ExternalInput")
    W1e = nc.dram_tensor("W1ext", [F_IN, 144], F16, kind="ExternalInput")
    W2e = nc.dram_tensor("W2ext", [128, 43], F16, kind="ExternalInput")
    IDX1 = nc.dram_tensor("IDX1", [128, t_total * 8], I16, kind="ExternalInput")
    IDXD = nc.dram_tensor("IDXD", [128, t_total * 8], I16, kind="ExternalInput")
    DSTI = nc.dram_tensor("DSTI", [128, t_total], I16, kind="ExternalInput")
    IOTAA = nc.dram_tensor("iotaa", [128, NPAD], I16, kind="ExternalInput")
    B1 = nc.dram_tensor("B1", [1, 128], F16, kind="ExternalInput")
    B2 = nc.dram_tensor("B2", [1, NCLASS], F16, kind="ExternalInput")
    OUT = nc.dram_tensor("out", [NPAD, NCLASS], F32, kind="ExternalOutput")

    tab1_sh = nc.dram_tensor("tab1_sh", [NPAD, ROW1], F16, kind="Internal")
    tab1 = nc.dram_tensor("tab1", [NPAD * NCORES, ROW1], F16, kind="Internal",
                          addr_space="Shared")
    tab2_sh = nc.dram_tensor("tab2_sh", [NPAD, U2], F16, kind="Internal")
    tab2c = nc.dram_tensor("tab2c", [NPAD * NCORES, U2], F16, kind="Internal",
                           addr_space="Shared")
    tab2 = nc.dram_tensor("tab2", [NPAD * NCORES, ROW2], F16, kind="Internal")
    adr1 = nc.dram_tensor("adr1", [NPAD, ROWA], F16, kind="Internal")
    adr2 = nc.dram_tensor("adr2", [NPAD, ROWA], F16, kind="Internal")

    eq = mybir.AluOpType.is_equal
    mult = mybir.AluOpType.mult
    amax = mybir.AluOpType.max
    aadd = mybir.AluOpType.add
    sub = mybir.AluOpType.subtract
    AF = mybir.ActivationFunctionType
    AX = mybir.AxisListType

    with tile.TileContext(nc) as tc:
        with tc.tile_pool(name="sbW", bufs=1) as sbw:
            iotaa = sbw.tile([128, NPAD], I16, tag="iotaa", name="iotaa")
            nc.sync.dma_start(iotaa[:], IOTAA[:])
            ident = sbw.tile([128, 128], F16, tag="ident", name="ident")
            make_identity(nc, ident[:])
            w2s = sbw.tile([128, 43], F16, tag="w2s", name="w2s")
            nc.sync.dma_start(w2s[:], W2e[:])
            b1t = sbw.tile([128, 128], F16, tag="b1t", name="b1t")
            nc.sync.dma_start(b1t[:], B1[:].to_broadcast([128, 128]))
            b2t = sbw.tile([128, NCLASS], F16, tag="b2t", name="b2t")
            nc.sync.dma_start(b2t[:], B2[:].to_broadcast([128, NCLASS]))

            selfacc = sbw.tile([128, NCH * U1], F16, tag="selfacc",
                               name="selfacc")
            self2acc = sbw.tile([128, NCH * (NCLASS + 1)], F16,
                                tag="self2acc", name="self2acc")
            if "A" in phases:
                _phase_a(nc, tc, xT, W1e, tab1_sh, adr1, selfacc, AF)
            if "B" in phases:
                nc.gpsimd.collective_compute(
                    "AllGather", mybir.AluOpType.bypass,
                    replica_groups=[list(range(NCORES))],
                    ins=[tab1_sh[:]], outs=[tab1[:]])
            with tc.tile_pool(name="sbAcc", bufs=1) as sbacc:
                h1acc = sbacc.tile([128, NCH * U1], F16, tag="h1acc",
                                   name="h1acc")
                h2acc = sbacc.tile([128, NCH * 43], F16, tag="h2acc",
                                   name="h2acc")
                oacc = sbacc.tile([128, NCH * (NCLASS + 1)], F16, tag="oacc",
                                  name="oacc")
                if "C" in phases:
                    _phase_c(nc, tc, shapes2, maxcnt2, ranges, g_tiles, g_off,
                             TGMAX, TCMAX, IDX1, IDXD, DSTI, iotaa, b1t, w2s,
                             tab1, adr1, h1acc, h2acc, tab2_sh, adr2,
                             selfacc, self2acc, eq, mult, amax, aadd, AF)
                if "D" in phases:
                    nc.gpsimd.collective_compute(
                        "AllGather", mybir.AluOpType.bypass,
                        replica_groups=[list(range(NCORES))],
                        ins=[tab2_sh[:]], outs=[tab2c[:]])
                    for k in range(NCORES):
                        nc.sync.dma_start(
                            tab2[k * NPAD:(k + 1) * NPAD, 0:U2],
                            tab2c[k * NPAD:(k + 1) * NPAD, :])
                if "E" in phases:
                    _phase_e(nc, tc, shapes2, maxcnt2, ranges, g_tiles, g_off,
                             TGMAX, TCMAX, IDX1, IDXD, DSTI, iotaa, b2t,
                             tab2, adr2, oacc, OUT, self2acc,
                             eq, mult, amax, aadd, sub, AF, AX)

    nc.compile()
    return nc


def _phase_a(nc, tc, xT, W1e, tab1_sh, adr1, selfacc, AF):
    mult = mybir.AluOpType.mult
    amax = mybir.AluOpType.max
    aadd = mybir.AluOpType.add
    with tc.tile_pool(name="sbA", bufs=1) as sba, \
         tc.tile_pool(name="psA", bufs=4, space="PSUM") as psa:
        xts = [sba.tile([128, NPAD], F16, tag=f"xt{k}", name=f"xt{k}")
               for k in range(4)]
        w1s = [sba.tile([128, 144], F16, tag=f"w1{k}", name=f"w1{k}")
               for k in range(4)]
        aacc = sba.tile([128, NCH * 144], F16, tag="aacc", name="aacc")
        for k in range(4):
            nc.sync.dma_start(xts[k][:], xT[k * 128:(k + 1) * 128, :])
            nc.sync.dma_start(w1s[k][:], W1e[k * 128:(k + 1) * 128, :])
        for nt in range(NCH):
            ps = psa.tile([128, 144], F32, tag="psA", name="psA")
            for k in range(4):
                nc.tensor.matmul(ps[:], lhsT=xts[k][:, nt * 128:(nt + 1) * 128],
                                 rhs=w1s[k][:], start=(k == 0), stop=(k == 3))
            nc.scalar.activation(out=aacc[:, nt * 144:(nt + 1) * 144],
                                 in_=ps[:], func=AF.Copy)
        a3 = aacc[:].rearrange("p (d e) -> p d e", e=144)
        nc.sync.dma_start(
            tab1_sh.rearrange("(d p) e -> p d e", p=128)[:, :, 0:U1],
            a3[:, :, 0:U1])
        nc.sync.dma_start(
            adr1.rearrange("(d p) e -> p d e", p=128)[:, :, 0:8],
            a3[:, :, 136:144])
        # node-local self-loop term: selfacc = [h1 * w_self | w_self],
        # w_self = exp(leaky(asrc + adst))
        sw = sba.tile([128, NCH * 8], F16, tag="sw", name="sw")
        sw3 = sw[:].rearrange("p (d h) -> p d h", h=8)
        nc.vector.tensor_tensor(out=sw3, in0=a3[:, :, 128:136],
                                in1=a3[:, :, 136:144], op=aadd)
        nc.vector.scalar_tensor_tensor(
            out=sw3, in0=sw3, scalar=NEG_SLOPE, in1=sw3, op0=mult, op1=amax)
        nc.scalar.activation(out=sw[:], in_=sw[:], func=AF.Exp)
        sf3 = selfacc[:].rearrange("p (d e) -> p d e", e=U1)
        nc.vector.tensor_tensor(
            out=sf3[:, :, 0:128].rearrange("p d (h c) -> p d h c", c=NHID),
            in0=a3[:, :, 0:128].rearrange("p d (h c) -> p d h c", c=NHID),
            in1=sw[:].rearrange("p (d h c) -> p d h c", h=8, c=1)
            .to_broadcast([128, NCH, 8, NHID]),
            op=mult)
        nc.vector.tensor_copy(sf3[:, :, 128:136], sw3)


def _chunk_tiles(ranges, shapes2, g, c4):
    """[(segment-local tile, ind-local tile)] per window for chunk (g, c4)."""
    segs = []
    indoff = 0
    loc = 0  # group-local tile offset of window s
    for s in range(NSCH):
        a, b = int(ranges[g, s, c4, 0]), int(ranges[g, s, c4, 1])
        if b > a:
            segs.append((s, loc + a, indoff, b - a))
            indoff += b - a
        loc += int(shapes2[g, s]) // 128
    return segs, indoff


def _phase_c(nc, tc, shapes2, maxcnt2, ranges, g_tiles, g_off, TGMAX, TCMAX,
             IDX1, IDXD, DSTI, iotaa, b1t, w2s, tab1, adr1,
             h1acc, h2acc, tab2_sh, adr2, selfacc, self2acc,
             eq, mult, amax, aadd, AF):
    ng = shapes2.shape[0]
    with tc.tile_pool(name="sbC", bufs=2) as sbg, \
         tc.tile_pool(name="psC", bufs=4, space="PSUM") as psc:
        for g in range(ng):
            Tg = g_tiles[g]
            if Tg == 0:
                continue
            goff = g_off[g]
            i1 = sbg.tile([128, TGMAX * 8], I16, tag="i1", name="i1")
            nc.sync.dma_start(i1[:, 0:Tg * 8], IDX1[:, goff * 8:(goff + Tg) * 8])
            idd = sbg.tile([128, TGMAX * 8], I16, tag="idd", name="idd")
            nc.sync.dma_start(idd[:, 0:Tg * 8], IDXD[:, goff * 8:(goff + Tg) * 8])
            dlc = sbg.tile([128, TGMAX], I16, tag="dlc", name="dlc")
            nc.sync.dma_start(dlc[:, 0:Tg], DSTI[:, goff:goff + Tg])

            g1 = sbg.tile([128, TGMAX * U1], F16, tag="g1", name="g1")
            ga = sbg.tile([128, TGMAX * 8], F16, tag="ga", name="ga")
            if g < 2:
                # first use of each rotating buffer: clear stale SBUF so the
                # never-gathered tail slots hold finite values
                nc.vector.memset(g1[:], 0.0)
                nc.vector.memset(ga[:], 0.0)
            loc = 0
            for s in range(NSCH):
                cl = int(shapes2[g, s])
                if cl == 0:
                    continue
                mx = int(maxcnt2[g, s])
                n16 = -(-mx // 16) * 16
                nt = -(-n16 // 128)
                _dma_gather_raw(
                    nc.gpsimd,
                    g1[:, (loc // 128) * U1:((loc // 128) + nt) * U1]
                    .rearrange("p (t e) -> p t e", e=U1),
                    tab1[s * SCHW:(s + 1) * SCHW, :],
                    i1[:, loc // 16:(loc + n16) // 16], n16, mx, U1, ROW1,
                    queue_num=s)
                _dma_gather_raw(
                    nc.gpsimd,
                    ga[:, (loc // 128) * 8:((loc // 128) + nt) * 8]
                    .rearrange("p (t e) -> p t e", e=8),
                    adr1[:],
                    idd[:, loc // 16:(loc + n16) // 16], n16, mx, 8, ROWA,
                    queue_num=s)
                loc += cl

            g13 = g1[:, 0:Tg * U1].rearrange("p (t e) -> p t e", e=U1)
            ga3 = ga[:, 0:Tg * 8].rearrange("p (t e) -> p t e", e=8)

            wst = sbg.tile([128, TGMAX * 8], F16, tag="wst", name="wst")
            w3 = wst[:, 0:Tg * 8].rearrange("p (t h) -> p t h", h=8)
            nc.vector.tensor_tensor(out=w3, in0=g13[:, :, 128:136],
                                    in1=ga3, op=aadd)
            nc.vector.scalar_tensor_tensor(
                out=w3, in0=w3, scalar=NEG_SLOPE, in1=w3, op0=mult, op1=amax)
            nc.scalar.activation(out=wst[:, 0:Tg * 8], in_=wst[:, 0:Tg * 8],
                                 func=AF.Exp)

            ust = sbg.tile([128, TGMAX * U1], F16, tag="ust", name="ust")
            nc.vector.tensor_tensor(
                out=ust[:, 0:Tg * U1].rearrange("p (t e) -> p t e", e=U1)
                [:, :, 0:128].rearrange("p t (h c) -> p t h c", c=NHID),
                in0=g13[:, :, 0:128].rearrange("p t (h c) -> p t h c", c=NHID),
                in1=wst[:, 0:Tg * 8].rearrange("p (t h c) -> p t h c", h=8, c=1)
                .to_broadcast([128, Tg, 8, NHID]),
                op=mult)
            nc.vector.tensor_copy(
                ust[:, 0:Tg * U1].rearrange("p (t e) -> p t e", e=U1)
                [:, :, 128:136], w3)

            for c4 in range(G1):
                d = g * G1 + c4
                if d >= NCH:
                    break
                segs, indlen = _chunk_tiles(ranges, shapes2, g, c4)
                if not segs:
                    nc.vector.tensor_copy(h1acc[:, d * U1:(d + 1) * U1],
                                          selfacc[:, d * U1:(d + 1) * U1])
                    continue
                ind = sbg.tile([128, TCMAX * 128], F16, tag="ind", name="ind")
                for (s, gloc, ioff, ntl) in segs:
                    nc.vector.tensor_tensor(
                        out=ind[:, ioff * 128:(ioff + ntl) * 128]
                        .rearrange("p (t s) -> p t s", s=128),
                        in0=dlc[:, gloc:gloc + ntl]
                        .rearrange("p (t s) -> p t s", s=1)
                        .to_broadcast([128, ntl, 128]),
                        in1=iotaa[:, d * 128:(d + 1) * 128]
                        .rearrange("p (t s) -> p t s", t=1)
                        .to_broadcast([128, ntl, 128]),
                        op=eq)
                ps1 = psc.tile([128, U1], F32, tag="ps1", name="ps1")
                j = 0
                ntot = sum(x[3] for x in segs)
                for (s, gloc, ioff, ntl) in segs:
                    for t in range(ntl):
                        nc.tensor.matmul(
                            ps1[:],
                            lhsT=ind[:, (ioff + t) * 128:(ioff + t + 1) * 128],
                            rhs=ust[:, (gloc + t) * U1:(gloc + t + 1) * U1],
                            start=(j == 0), stop=(j == ntot - 1))
                        j += 1
                nc.vector.tensor_tensor(
                    out=h1acc[:, d * U1:(d + 1) * U1], in0=ps1[:],
                    in1=selfacc[:, d * U1:(d + 1) * U1], op=aadd)

    with tc.tile_pool(name="sbC2", bufs=1) as sb2, \
         tc.tile_pool(name="psC2", bufs=2, space="PSUM") as ps2p:
        h3 = h1acc[:].rearrange("p (d e) -> p d e", e=U1)
        rc = sb2.tile([128, NCH * 8], F16, tag="rc", name="rc")
        rc3 = rc[:].rearrange("p (d h) -> p d h", h=8)
        with nc.allow_low_precision(reason="fp16 recip of O(100) softmax sums"):
            nc.vector.reciprocal(rc3, h3[:, :, 128:136])
        o1 = sb2.tile([128, NCH * 128], F16, tag="o1", name="o1")
        nc.vector.tensor_tensor(
            out=o1[:].rearrange("p (d h c) -> p d h c", h=8, c=NHID),
            in0=h3[:, :, 0:128].rearrange("p d (h c) -> p d h c", c=NHID),
            in1=rc[:].rearrange("p (d h c) -> p d h c", h=8, c=1)
            .to_broadcast([128, NCH, 8, NHID]),
            op=mult)
        o3 = o1[:].rearrange("p (d e) -> p d e", e=128)
        nc.vector.tensor_tensor(
            out=o3, in0=o3,
            in1=b1t[:].rearrange("p (d e) -> p d e", d=1)
            .to_broadcast([128, NCH, 128]),
            op=aadd)
        t2 = sb2.tile([128, NCH * 128], F16, tag="t2", name="t2")
        nc.vector.tensor_scalar_min(t2[:], o1[:], 0.0)
        nc.scalar.activation(out=t2[:], in_=t2[:], func=AF.Exp)
        nc.vector.tensor_scalar_add(t2[:], t2[:], -1.0)
        nc.vector.tensor_scalar_max(o1[:], o1[:], 0.0)
        nc.vector.tensor_tensor(out=o1[:], in0=o1[:], in1=t2[:], op=aadd)

        from concourse.masks import make_identity
        identt = sb2.tile([128, 128], F16, tag="id2", name="id2")
        make_identity(nc, identt[:])
        for d in range(NCH):
            psT = ps2p.tile([128, 128], F16, tag="psT", name="psT")
            nc.tensor.transpose(psT[:], o1[:, d * 128:(d + 1) * 128], identt[:])
            eluT = sb2.tile([128, 128], F16, tag="eluT", name="eluT")
            nc.scalar.activation(out=eluT[:], in_=psT[:], func=AF.Copy)
            ps2a = ps2p.tile([128, 43], F32, tag="ps2a", name="ps2a")
            nc.tensor.matmul(ps2a[:], lhsT=eluT[:], rhs=w2s[:],
                             start=True, stop=True)
            nc.scalar.activation(out=h2acc[:, d * 43:(d + 1) * 43],
                                 in_=ps2a[:], func=AF.Copy)
        hh3 = h2acc[:].rearrange("p (d e) -> p d e", e=43)
        sw2 = sb2.tile([128, NCH], F16, tag="sw2", name="sw2")
        sw23 = sw2[:].rearrange("p (d h) -> p d h", h=1)
        nc.vector.tensor_tensor(out=sw23, in0=hh3[:, :, 41:42],
                                in1=hh3[:, :, 42:43], op=aadd)
        nc.vector.scalar_tensor_tensor(
            out=sw23, in0=sw23, scalar=NEG_SLOPE, in1=sw23,
            op0=mult, op1=amax)
        nc.scalar.activation(out=sw2[:], in_=sw2[:], func=AF.Exp)
        s23 = self2acc[:].rearrange("p (d e) -> p d e", e=NCLASS + 1)
        nc.vector.tensor_tensor(
            out=s23[:, :, 0:NCLASS], in0=hh3[:, :, 0:NCLASS],
            in1=sw23.to_broadcast([128, NCH, NCLASS]), op=mult)
        nc.vector.tensor_copy(s23[:, :, NCLASS:NCLASS + 1], sw23)
        nc.vector.memset(hh3[:, :, 40:41], 1.0)
        nc.sync.dma_start(
            tab2_sh.rearrange("(d p) e -> p d e", p=128),
            hh3[:, :, 0:U2])
        nc.sync.dma_start(
            adr2.rearrange("(d p) e -> p d e", p=128)[:, :, 0:1],
            hh3[:, :, 42:43])


def _phase_e(nc, tc, shapes2, maxcnt2, ranges, g_tiles, g_off, TGMAX, TCMAX,
             IDX1, IDXD, DSTI, iotaa, b2t, tab2, adr2, oacc, OUT, self2acc,
             eq, mult, amax, aadd, sub, AF, AX):
    ng = shapes2.shape[0]
    NC1 = NCLASS + 1
    with tc.tile_pool(name="sbE", bufs=2) as sbg, \
         tc.tile_pool(name="psE", bufs=4, space="PSUM") as pse:
        for g in range(ng):
            Tg = g_tiles[g]
            if Tg == 0:
                continue
            goff = g_off[g]
            i1 = sbg.tile([128, TGMAX * 8], I16, tag="i1e", name="i1e")
            nc.sync.dma_start(i1[:, 0:Tg * 8], IDX1[:, goff * 8:(goff + Tg) * 8])
            idd = sbg.tile([128, TGMAX * 8], I16, tag="idde", name="idde")
            nc.sync.dma_start(idd[:, 0:Tg * 8], IDXD[:, goff * 8:(goff + Tg) * 8])
            dlc = sbg.tile([128, TGMAX], I16, tag="dlce", name="dlce")
            nc.sync.dma_start(dlc[:, 0:Tg], DSTI[:, goff:goff + Tg])

            g2 = sbg.tile([128, TGMAX * U2], F16, tag="g2", name="g2")
            ga2 = sbg.tile([128, TGMAX * 8], F16, tag="ga2", name="ga2")
            if g < 2:
                nc.vector.memset(g2[:], 0.0)
                nc.vector.memset(ga2[:], 0.0)
            loc = 0
            for s in range(NSCH):
                cl = int(shapes2[g, s])
                if cl == 0:
                    continue
                mx = int(maxcnt2[g, s])
                n16 = -(-mx // 16) * 16
                nt = -(-n16 // 128)
                _dma_gather_raw(
                    nc.gpsimd,
                    g2[:, (loc // 128) * U2:((loc // 128) + nt) * U2]
                    .rearrange("p (t e) -> p t e", e=U2),
                    tab2[s * SCHW:(s + 1) * SCHW, :],
                    i1[:, loc // 16:(loc + n16) // 16], n16, mx, U2, ROW2,
                    queue_num=s)
                _dma_gather_raw(
                    nc.gpsimd,
                    ga2[:, (loc // 128) * 8:((loc // 128) + nt) * 8]
                    .rearrange("p (t e) -> p t e", e=8),
                    adr2[:],
                    idd[:, loc // 16:(loc + n16) // 16], n16, mx, 8, ROWA,
                    queue_num=s)
                loc += cl

            g23 = g2[:, 0:Tg * U2].rearrange("p (t e) -> p t e", e=U2)
            ga23 = ga2[:, 0:Tg * 8].rearrange("p (t e) -> p t e", e=8)

            w2t = sbg.tile([128, TGMAX], F16, tag="w2t", name="w2t")
            wt3 = w2t[:, 0:Tg].rearrange("p (t h) -> p t h", h=1)
            nc.vector.tensor_tensor(out=wt3, in0=g23[:, :, 41:42],
                                    in1=ga23[:, :, 0:1], op=aadd)
            nc.vector.scalar_tensor_tensor(
                out=wt3, in0=wt3, scalar=NEG_SLOPE, in1=wt3,
                op0=mult, op1=amax)
            nc.scalar.activation(out=w2t[:, 0:Tg], in_=w2t[:, 0:Tg],
                                 func=AF.Exp)

            gw = sbg.tile([128, TGMAX * NC1], F16, tag="gw", name="gw")
            nc.vector.tensor_tensor(
                out=gw[:, 0:Tg * NC1].rearrange("p (t e) -> p t e", e=NC1),
                in0=g23[:, :, 0:NC1],
                in1=w2t[:, 0:Tg].rearrange("p (t s) -> p t s", s=1)
                .to_broadcast([128, Tg, NC1]),
                op=mult)

            for c4 in range(G1):
                d = g * G1 + c4
                if d >= NCH:
                    break
                segs, indlen = _chunk_tiles(ranges, shapes2, g, c4)
                if not segs:
                    nc.vector.tensor_copy(oacc[:, d * NC1:(d + 1) * NC1],
                                          self2acc[:, d * NC1:(d + 1) * NC1])
                    continue
                ind = sbg.tile([128, TCMAX * 128], F16, tag="inde", name="inde")
                for (s, gloc, ioff, ntl) in segs:
                    nc.vector.tensor_tensor(
                        out=ind[:, ioff * 128:(ioff + ntl) * 128]
                        .rearrange("p (t s) -> p t s", s=128),
                        in0=dlc[:, gloc:gloc + ntl]
                        .rearrange("p (t s) -> p t s", s=1)
                        .to_broadcast([128, ntl, 128]),
                        in1=iotaa[:, d * 128:(d + 1) * 128]
                        .rearrange("p (t s) -> p t s", t=1)
                        .to_broadcast([128, ntl, 128]),
                        op=eq)
                ps2 = pse.tile([128, NC1], F32, tag="ps2", name="ps2")
                j = 0
                ntot = sum(x[3] for x in segs)
                for (s, gloc, ioff, ntl) in segs:
                    for t in range(ntl):
                        nc.tensor.matmul(
                            ps2[:],
                            lhsT=ind[:, (ioff + t) * 128:(ioff + t + 1) * 128],
                            rhs=gw[:, (gloc + t) * NC1:(gloc + t + 1) * NC1],
                            start=(j == 0), stop=(j == ntot - 1))
                        j += 1
                nc.vector.tensor_tensor(
                    out=oacc[:, d * NC1:(d + 1) * NC1], in0=ps2[:],
                    in1=self2acc[:, d * NC1:(d + 1) * NC1], op=aadd)

    with tc.tile_pool(name="sbE2", bufs=1) as sb2:
        oa3 = oacc[:].rearrange("p (d e) -> p d e", e=NC1)
        rc2 = sb2.tile([128, NCH], F16, tag="rc2", name="rc2")
        with nc.allow_low_precision(reason="fp16 recip of O(100) softmax sums"):
            nc.vector.reciprocal(rc2[:].rearrange("p (d h) -> p d h", h=1),
                                 oa3[:, :, NCLASS:NC1])
        lg = sb2.tile([128, NCH * NCLASS], F32, tag="lg", name="lg")
        lg3 = lg[:].rearrange("p (d e) -> p d e", e=NCLASS)
        nc.vector.tensor_tensor(
            out=lg3, in0=oa3[:, :, 0:NCLASS],
            in1=rc2[:].rearrange("p (d h) -> p d h", h=1)
            .to_broadcast([128, NCH, NCLASS]),
            op=mult)
        nc.vector.tensor_tensor(
            out=lg3, in0=lg3,
            in1=b2t[:].rearrange("p (d e) -> p d e", d=1)
            .to_broadcast([128, NCH, NCLASS]),
            op=aadd)
        ex = sb2.tile([128, NCH * NCLASS], F32, tag="ex", name="ex")
        nc.scalar.activation(out=ex[:], in_=lg[:], func=AF.Exp)
        sm = sb2.tile([128, NCH], F32, tag="sm", name="sm")
        nc.vector.tensor_reduce(
            out=sm[:], in_=ex[:].rearrange("p (d e) -> p d e", e=NCLASS),
            axis=AX.X, op=aadd)
        ln = sb2.tile([128, NCH], F32, tag="ln", name="ln")
        nc.scalar.activation(out=ln[:], in_=sm[:], func=AF.Ln)
        nc.vector.tensor_tensor(
            out=lg3, in0=lg3,
            in1=ln[:].rearrange("p (d h) -> p d h", h=1)
            .to_broadcast([128, NCH, NCLASS]),
            op=sub)
        nc.sync.dma_start(
            OUT.rearrange("(d p) e -> p d e", p=128), lg3)


_CACHE = {}


def kernel(x, edge_index, W1, att_src1, att_dst1, b1, W2, att_src2, att_dst2, b2):
    x = np.asarray(x, dtype=np.float32)
    edge_index = np.asarray(edge_index)
    in_maps, meta = _prep(np.asarray(x), edge_index,
                          np.asarray(W1), np.asarray(att_src1),
                          np.asarray(att_dst1), np.asarray(W2),
                          np.asarray(att_src2), np.asarray(att_dst2),
                          b1=b1, b2=b2)
    key = (meta[0].tobytes(), meta[1].tobytes(), meta[2].tobytes())
    if key not in _CACHE:
        _CACHE[key] = _build(meta)
    nc = _CACHE[key]
    res = run_bass_kernel_spmd(nc, in_maps, core_ids=list(range(NCORES)))
    out = np.concatenate([res.results[k]["out"][:NPC] for k in range(NCORES)],
                         axis=0)
    return out.astype(np.float32)


# revision 4
# speedup vs baseline: 2.3613x; 1.1156x over previous
#!/usr/bin/env python3
"""2-layer GAT on 8 NeuronCores (Bass/Tile).

Sharding: nodes partitioned across 8 cores by dst id (graph parallel),
small weight/attention params replicated, per-node feature tables
allgathered, per-edge source rows fetched with dma_gather, segment
softmax/aggregation via indicator matmuls on the tensor engine.

HW dma_gather cost is dominated by a serial per-descriptor cost, so the
layout minimizes gather descriptors: edges are packed per (group, window) segment
(sorted by chunk within the segment) and padded only at the segment tail,
instead of 128-padding every (chunk, window) cell. Descriptor streams are
trimmed to the cross-core max valid count per segment (dummy idx-0 rows up
to the max keep the SPMD descriptor count identical on every core, then a
trailing run of -1 indices that the DGE skips). Tiles may straddle chunk
boundaries; the indicator is built per (chunk, window) tile range with an
absolute int16 dst-id compare, so foreign edges in a shared tile hit no
indicator column. Cuts ~25% of gather descriptors, DVE elements, and PE
matmul tiles. Self-loop contributions are computed node-locally (no
gather at all), which also removes the per-core self-loop window skew
from the cross-core max. Normalization + bias + ELU + log-softmax run as
batched DVE/Act sweeps; the layer-2 allgather ships the compact
[NPAD,42] table and spreads it locally into the 256B-stride gather
table.
"""
import sys
import numpy as np

sys.path.insert(0, "/opt/pypackages")
sys.path.insert(0, "/opt/trn_rl_repo")

import concourse.bass as bass
import concourse.bacc as bacc
import concourse.tile as tile
import concourse.mybir as mybir
from concourse.bass_utils import run_bass_kernel_spmd

# problem constants
N = 100000
F_IN = 512
NHID = 16
HEADS = 8
NCLASS = 40
E = 1600000
NEG_SLOPE = 0.2

NCORES = 8
NPC = N // NCORES
DCH = 128
NCH = (NPC + DCH - 1) // DCH
NPAD = NCH * DCH             # 12544
NSCH = 4
SCHW = (NPAD * NCORES) // NSCH  # 25088
G1 = 3
NG = (NCH + G1 - 1) // G1

ROW1 = 256
U1 = 136
ROW2 = 128
U2 = 42
ROWA = 128

F16 = mybir.dt.float16
F32 = mybir.dt.float32
I16 = mybir.dt.int16


def _wrap_block(v):
    w = v.reshape(-1, 16).T
    return np.tile(w, (8, 1))


def _dma_gather_raw(gp, out_ap, in_ap, idxs_ap, num_idxs, num_valid,
                    elem_size, elem_step, queue_num=0):
    """dma_gather with elem_size not a multiple of 256B and explicit valid
    count (trailing -1 indices are skipped by the DGE)."""
    from concourse.bass import exact_div
    stride_bytes = elem_step * mybir.dt.size(in_ap.dtype)
    stride_bytes_256 = exact_div(stride_bytes, 256)
    _in_ap = gp.lower_ap_dma(in_ap, for_custom_bir_dma=True)
    _idxs_ap = gp.lower_ap(idxs_ap)
    _out_ap = gp.lower_ap(out_ap)
    return gp.add_instruction(
        mybir.InstDMAGatherAnt(
            name=gp.bass.get_next_instruction_name(),
            ins=[*_in_ap, _idxs_ap, gp.lower_val_access(gp.to_reg(num_valid))],
            outs=[_out_ap],
            transpose=False, num_idxs=num_idxs, elem_size=elem_size,
            stride_bytes_256=stride_bytes_256, gen_mode=0,
            single_packet=False, queue_num=queue_num,
            sbuf_tokens_per_rank=0, sbuf_free_dim_per_rank=0,
            sbuf_free_dim_pad_per_rank=0, sbuf_byte_offset=0))


def _prep(x, edge_index, W1, att_src1, att_dst1, W2, att_src2, att_dst2,
          b1=None, b2=None):
    """Returns (in_maps, meta) where meta carries the static structure
    shared by all cores: seg_shapes [NG,NSCH] (128-rounded segment lens),
    seg_maxcnt [NG,NSCH] (cross-core max valid count), and ranges
    [NCH,NSCH,2] (tile [a,b) of each chunk within its segment)."""
    # self-loop contributions are computed node-locally on device (no
    # gather), so the edge stream holds only the real edges
    src = np.asarray(edge_index[0], dtype=np.int64)
    dst = np.asarray(edge_index[1], dtype=np.int64)

    core = dst // NPC
    dl = (dst - core * NPC).astype(np.int64)
    dch = dl >> 7
    gg = dch // G1
    c4 = dch % G1
    s_pad = (src // NPC) * NPAD + (src % NPC)
    sch = s_pad // SCHW
    sloc = (s_pad - sch * SCHW).astype(np.int64)

    seg = (core * NG + gg) * NSCH + sch             # (core, g, s) segment id
    sub = c4 * (SCHW + 1) + sloc                    # order by chunk then src
    order = np.argsort(seg * (G1 * (SCHW + 1) + 2) + sub, kind="stable")
    seg_s, sloc_s, dl_s = seg[order], sloc[order], dl[order]
    c4_s = c4[order]

    nseg = NG * NSCH
    counts = np.bincount(seg_s, minlength=NCORES * nseg).reshape(NCORES, nseg)
    seg_maxcnt = counts.max(axis=0)                          # [nseg]
    seg_shapes = (np.ceil(seg_maxcnt / 128.0).astype(np.int64) * 128)
    seg_starts = np.concatenate([[0], np.cumsum(seg_shapes)])
    t_total = int(seg_starts[-1]) // 128

    # per-(core, g, s, c4) counts for the static chunk tile ranges
    cell4 = seg_s * G1 + c4_s
    counts4 = np.bincount(cell4, minlength=NCORES * nseg * G1) \
        .reshape(NCORES, nseg, G1)
    pre4 = np.concatenate([np.zeros((NCORES, nseg, 1), np.int64),
                           np.cumsum(counts4, axis=2)], axis=2)
    a4 = pre4[:, :, :G1].min(axis=0) // 128                  # [nseg, G1]
    b4 = -(-pre4[:, :, 1:].max(axis=0) // 128)               # ceil
    b4 = np.minimum(b4, (seg_shapes // 128)[:, None])
    a4 = np.minimum(a4, b4)

    seg_first = np.concatenate([[0], np.cumsum(counts.reshape(-1))])
    rank = np.arange(len(seg_s)) - seg_first[seg_s]
    pos = seg_starts[seg_s % nseg] + rank
    core_s = seg_s // nseg

    L = t_total * 128
    # default -1 (skipped); dummy 0 up to maxcnt set below
    idx1 = np.full((NCORES, L), -1, dtype=np.int16)
    idxd = np.full((NCORES, L), -1, dtype=np.int16)
    dsti = np.full((NCORES, L), -1, dtype=np.int16)
    idx1[core_s, pos] = sloc_s.astype(np.int16)
    idxd[core_s, pos] = dl_s.astype(np.int16)
    dsti[core_s, pos] = dl_s.astype(np.int16)
    # dummy idx-0 rows in [count_core, maxcnt) keep the descriptor count
    # static across cores (dsti stays -1 so they hit no indicator column)
    for k in range(NCORES):
        for sg in range(nseg):
            lo = seg_starts[sg] + counts[k, sg]
            hi = seg_starts[sg] + seg_maxcnt[sg]
            idx1[k, lo:hi] = 0
            idxd[k, lo:hi] = 0

    shapes2 = seg_shapes.reshape(NG, NSCH)
    maxcnt2 = seg_maxcnt.reshape(NG, NSCH)
    ranges = np.stack([a4, b4], axis=2).reshape(NG, NSCH, G1, 2)

    IDX1 = np.zeros((NCORES, 128, L // 16), dtype=np.int16)
    IDXD = np.zeros((NCORES, 128, L // 16), dtype=np.int16)
    for k in range(NCORES):
        off = 0
        for g in range(NG):
            for s in range(NSCH):
                cl = int(shapes2[g, s])
                if cl:
                    IDX1[k][:, off // 16:(off + cl) // 16] = \
                        _wrap_block(idx1[k, off:off + cl])
                    IDXD[k][:, off // 16:(off + cl) // 16] = \
                        _wrap_block(idxd[k, off:off + cl])
                off += cl
    DSTI = dsti.reshape(NCORES, t_total, 128).transpose(0, 2, 1).copy()

    asrc1 = att_src1.reshape(HEADS, NHID)
    adst1 = att_dst1.reshape(HEADS, NHID)
    W1r = W1.reshape(F_IN, HEADS, NHID)
    W1as = np.einsum("khc,hc->kh", W1r, asrc1)
    W1ad = np.einsum("khc,hc->kh", W1r, adst1)
    W1ext = np.concatenate([W1, W1as, W1ad], axis=1).astype(np.float16)
    W2as = W2 @ att_src2.reshape(NCLASS, 1)
    W2ad = W2 @ att_dst2.reshape(NCLASS, 1)
    W2ext = np.concatenate([W2, np.zeros((HEADS * NHID, 1)), W2as, W2ad],
                           axis=1).astype(np.float16)

    iotaa = np.broadcast_to(np.arange(NPAD, dtype=np.int16), (128, NPAD)).copy()

    in_maps = []
    for k in range(NCORES):
        xs = x[k * NPC:(k + 1) * NPC]
        xT = np.zeros((F_IN, NPAD), dtype=np.float16)
        xT[:, :NPC] = xs.T
        in_maps.append({
            "xT": xT,
            "W1ext": W1ext,
            "W2ext": W2ext,
            "IDX1": IDX1[k],
            "IDXD": IDXD[k],
            "DSTI": DSTI[k],
            "iotaa": iotaa,
            "B1": (np.zeros((1, 128), np.float16) if b1 is None
                   else np.asarray(b1, np.float16).reshape(1, 128)),
            "B2": (np.zeros((1, NCLASS), np.float16) if b2 is None
                   else np.asarray(b2, np.float16).reshape(1, NCLASS)),
        })
    return in_maps, (shapes2, maxcnt2, ranges)


def _build(meta, phases="ABCDE"):
    from concourse.masks import make_identity
    shapes2, maxcnt2, ranges = meta
    shapes2 = np.asarray(shapes2)
    maxcnt2 = np.asarray(maxcnt2)
    ranges = np.asarray(ranges)
    ng = shapes2.shape[0]
    g_tiles = [int(shapes2[g].sum()) // 128 for g in range(ng)]
    t_total = sum(g_tiles)
    TGMAX = max(g_tiles)
    g_off = np.concatenate([[0], np.cumsum(g_tiles)]).astype(int)
    # max per-chunk indicator tiles
    TCMAX = int((ranges[:, :, :, 1] - ranges[:, :, :, 0]).sum(axis=1).max())

    nc = bacc.Bacc("TRN2", target_bir_lowering=False, debug=False,
                   enable_asserts=False, num_devices=NCORES,
                   num_swdge_queues=4)

    xT = nc.dram_tensor("xT", [F_IN, NPAD], F16, kind="ExternalInput")
    W1e = nc.dram_tensor("W1ext", [F_IN, 144], F16, kind="ExternalInput")
    W2e = nc.dram_tensor("W2ext", [128, 43], F16, kind="ExternalInput")
    IDX1 = nc.dram_tensor("IDX1", [128, t_total * 8], I16, kind="ExternalInput")
    IDXD = nc.dram_tensor("IDXD", [128, t_total * 8], I16, kind="ExternalInput")
    DSTI = nc.dram_tensor("DSTI", [128, t_total], I16, kind="ExternalInput")
    IOTAA = nc.dram_tensor("iotaa", [128, NPAD], I16, kind="ExternalInput")
    B1 = nc.dram_tensor("B1", [1, 128], F16, kind="ExternalInput")
    B2 = nc.dram_tensor("B2", [1, NCLASS], F16, kind="ExternalInput")
    OUT = nc.dram_tensor("out", [NPAD, NCLASS], F32, kind="ExternalOutput")

    tab1_sh = nc.dram_tensor("tab1_sh", [NPAD, ROW1], F16, kind="Internal")
    tab1 = nc.dram_tensor("tab1", [NPAD * NCORES, ROW1], F16, kind="Internal",
                          addr_space="Shared")
    tab2_sh = nc.dram_tensor("tab2_sh", [NPAD, U2], F16, kind="Internal")
    tab2c = nc.dram_tensor("tab2c", [NPAD * NCORES, U2], F16, kind="Internal",
                           addr_space="Shared")
    tab2 = nc.dram_tensor("tab2", [NPAD * NCORES, ROW2], F16, kind="Internal")
    adr1 = nc.dram_tensor("adr1", [NPAD, ROWA], F16, kind="Internal")
    adr2 = nc.dram_tensor("adr2", [NPAD, ROWA], F16, kind="Internal")

    eq = mybir.AluOpType.is_equal
    mult = mybir.AluOpType.mult
    amax = mybir.AluOpType.max
    aadd = mybir.AluOpType.add
    sub = mybir.AluOpType.subtract
    AF = mybir.ActivationFunctionType
    AX = mybir.AxisListType

    with tile.TileContext(nc) as tc:
        with tc.tile_pool(name="sbW", bufs=1) as sbw:
            iotaa = sbw.tile([128, NPAD], I16, tag="iotaa", name="iotaa")
            nc.sync.dma_start(iotaa[:], IOTAA[:])
            ident = sbw.tile([128, 128], F16, tag="ident", name="ident")
            make_identity(nc, ident[:])
            w2s = sbw.tile([128, 43], F16, tag="w2s", name="w2s")
            nc.sync.dma_start(w2s[:], W2e[:])
            b1t = sbw.tile([128, 128], F16, tag="b1t", name="b1t")
            nc.sync.dma_start(b1t[:], B1[:].to_broadcast([128, 128]))
            b2t = sbw.tile([128, NCLASS], F16, tag="b2t", name="b2t")
            nc.sync.dma_start(b2t[:], B2[:].to_broadcast([128, NCLASS]))

            selfacc = sbw.tile([128, NCH * U1], F16, tag="selfacc",
                               name="selfacc")
            self2acc = sbw.tile([128, NCH * (NCLASS + 1)], F16,
                                tag="self2acc", name="self2acc")
            if "A" in phases:
                _phase_a(nc, tc, xT, W1e, tab1_sh, adr1, selfacc, AF)
            if "B" in phases:
                nc.gpsimd.collective_compute(
                    "AllGather", mybir.AluOpType.bypass,
                    replica_groups=[list(range(NCORES))],
                    ins=[tab1_sh[:]], outs=[tab1[:]])
            with tc.tile_pool(name="sbAcc", bufs=1) as sbacc:
                h1acc = sbacc.tile([128, NCH * U1], F16, tag="h1acc",
                                   name="h1acc")
                h2acc = sbacc.tile([128, NCH * 43], F16, tag="h2acc",
                                   name="h2acc")
                oacc = sbacc.tile([128, NCH * (NCLASS + 1)], F16, tag="oacc",
                                  name="oacc")
                if "C" in phases:
                    _phase_c(nc, tc, shapes2, maxcnt2, ranges, g_tiles, g_off,
                             TGMAX, TCMAX, IDX1, IDXD, DSTI, iotaa, b1t, w2s,
                             tab1, adr1, h1acc, h2acc, tab2_sh, adr2,
                             selfacc, self2acc, eq, mult, amax, aadd, AF)
                if "D" in phases:
                    nc.gpsimd.collective_compute(
                        "AllGather", mybir.AluOpType.bypass,
                        replica_groups=[list(range(NCORES))],
                        ins=[tab2_sh[:]], outs=[tab2c[:]])
                    for k in range(NCORES):
                        nc.sync.dma_start(
                            tab2[k * NPAD:(k + 1) * NPAD, 0:U2],
                            tab2c[k * NPAD:(k + 1) * NPAD, :])
                if "E" in phases:
                    _phase_e(nc, tc, shapes2, maxcnt2, ranges, g_tiles, g_off,
                             TGMAX, TCMAX, IDX1, IDXD, DSTI, iotaa, b2t,
                             tab2, adr2, oacc, OUT, self2acc,
                             eq, mult, amax, aadd, sub, AF, AX)

    nc.compile()
    return nc


def _phase_a(nc, tc, xT, W1e, tab1_sh, adr1, selfacc, AF):
    mult = mybir.AluOpType.mult
    amax = mybir.AluOpType.max
    aadd = mybir.AluOpType.add
    with tc.tile_pool(name="sbA", bufs=1) as sba, \
         tc.tile_pool(name="psA", bufs=4, space="PSUM") as psa:
        xts = [sba.tile([128, NPAD], F16, tag=f"xt{k}", name=f"xt{k}")
               for k in range(4)]
        w1s = [sba.tile([128, 144], F16, tag=f"w1{k}", name=f"w1{k}")
               for k in range(4)]
        aacc = sba.tile([128, NCH * 144], F16, tag="aacc", name="aacc")
        for k in range(4):
            nc.sync.dma_start(xts[k][:], xT[k * 128:(k + 1) * 128, :])
            nc.sync.dma_start(w1s[k][:], W1e[k * 128:(k + 1) * 128, :])
        for nt in range(NCH):
            ps = psa.tile([128, 144], F32, tag="psA", name="psA")
            for k in range(4):
                nc.tensor.matmul(ps[:], lhsT=xts[k][:, nt * 128:(nt + 1) * 128],
                                 rhs=w1s[k][:], start=(k == 0), stop=(k == 3))
            nc.scalar.activation(out=aacc[:, nt * 144:(nt + 1) * 144],
                                 in_=ps[:], func=AF.Copy)
        a3 = aacc[:].rearrange("p (d e) -> p d e", e=144)
        nc.sync.dma_start(
            tab1_sh.rearrange("(d p) e -> p d e", p=128)[:, :, 0:U1],
            a3[:, :, 0:U1])
        nc.sync.dma_start(
            adr1.rearrange("(d p) e -> p d e", p=128)[:, :, 0:8],
            a3[:, :, 136:144])
        # node-local self-loop term: selfacc = [h1 * w_self | w_self],
        # w_self = exp(leaky(asrc + adst))
        sw = sba.tile([128, NCH * 8], F16, tag="sw", name="sw")
        sw3 = sw[:].rearrange("p (d h) -> p d h", h=8)
        nc.vector.tensor_tensor(out=sw3, in0=a3[:, :, 128:136],
                                in1=a3[:, :, 136:144], op=aadd)
        nc.vector.scalar_tensor_tensor(
            out=sw3, in0=sw3, scalar=NEG_SLOPE, in1=sw3, op0=mult, op1=amax)
        nc.scalar.activation(out=sw[:], in_=sw[:], func=AF.Exp)
        sf3 = selfacc[:].rearrange("p (d e) -> p d e", e=U1)
        nc.vector.tensor_tensor(
            out=sf3[:, :, 0:128].rearrange("p d (h c) -> p d h c", c=NHID),
            in0=a3[:, :, 0:128].rearrange("p d (h c) -> p d h c", c=NHID),
            in1=sw[:].rearrange("p (d h c) -> p d h c", h=8, c=1)
            .to_broadcast([128, NCH, 8, NHID]),
            op=mult)
        nc.vector.tensor_copy(sf3[:, :, 128:136], sw3)


def _chunk_tiles(ranges, shapes2, g, c4):
    """[(segment-local tile, ind-local tile)] per window for chunk (g, c4)."""
    segs = []
    indoff = 0
    loc = 0  # group-local tile offset of window s
    for s in range(NSCH):
        a, b = int(ranges[g, s, c4, 0]), int(ranges[g, s, c4, 1])
        if b > a:
            segs.append((s, loc + a, indoff, b - a))
            indoff += b - a
        loc += int(shapes2[g, s]) // 128
    return segs, indoff


def _phase_c(nc, tc, shapes2, maxcnt2, ranges, g_tiles, g_off, TGMAX, TCMAX,
             IDX1, IDXD, DSTI, iotaa, b1t, w2s, tab1, adr1,
             h1acc, h2acc, tab2_sh, adr2, selfacc, self2acc,
             eq, mult, amax, aadd, AF):
    ng = shapes2.shape[0]
    with tc.tile_pool(name="sbC", bufs=2) as sbg, \
         tc.tile_pool(name="psC", bufs=4, space="PSUM") as psc:
        for g in range(ng):
            Tg = g_tiles[g]
            if Tg == 0:
                continue
            goff = g_off[g]
            i1 = sbg.tile([128, TGMAX * 8], I16, tag="i1", name="i1")
            nc.sync.dma_start(i1[:, 0:Tg * 8], IDX1[:, goff * 8:(goff + Tg) * 8])
            idd = sbg.tile([128, TGMAX * 8], I16, tag="idd", name="idd")
            nc.sync.dma_start(idd[:, 0:Tg * 8], IDXD[:, goff * 8:(goff + Tg) * 8])
            dlc = sbg.tile([128, TGMAX], I16, tag="dlc", name="dlc")
            nc.sync.dma_start(dlc[:, 0:Tg], DSTI[:, goff:goff + Tg])

            g1 = sbg.tile([128, TGMAX * U1], F16, tag="g1", name="g1")
            ga = sbg.tile([128, TGMAX * 8], F16, tag="ga", name="ga")
            if g < 2:
                # first use of each rotating buffer: clear stale SBUF so the
                # never-gathered tail slots hold finite values
                nc.vector.memset(g1[:], 0.0)
                nc.vector.memset(ga[:], 0.0)
            loc = 0
            for s in range(NSCH):
                cl = int(shapes2[g, s])
                if cl == 0:
                    continue
                mx = int(maxcnt2[g, s])
                n16 = -(-mx // 16) * 16
                nt = -(-n16 // 128)
                _dma_gather_raw(
                    nc.gpsimd,
                    g1[:, (loc // 128) * U1:((loc // 128) + nt) * U1]
                    .rearrange("p (t e) -> p t e", e=U1),
                    tab1[s * SCHW:(s + 1) * SCHW, :],
                    i1[:, loc // 16:(loc + n16) // 16], n16, mx, U1, ROW1,
                    queue_num=s)
                _dma_gather_raw(
                    nc.gpsimd,
                    ga[:, (loc // 128) * 8:((loc // 128) + nt) * 8]
                    .rearrange("p (t e) -> p t e", e=8),
                    adr1[:],
                    idd[:, loc // 16:(loc + n16) // 16], n16, mx, 8, ROWA,
                    queue_num=(s + 2) % 4)
                loc += cl

            g13 = g1[:, 0:Tg * U1].rearrange("p (t e) -> p t e", e=U1)
            ga3 = ga[:, 0:Tg * 8].rearrange("p (t e) -> p t e", e=8)

            wst = sbg.tile([128, TGMAX * 8], F16, tag="wst", name="wst")
            w3 = wst[:, 0:Tg * 8].rearrange("p (t h) -> p t h", h=8)
            nc.vector.tensor_tensor(out=w3, in0=g13[:, :, 128:136],
                                    in1=ga3, op=aadd)
            nc.vector.scalar_tensor_tensor(
                out=w3, in0=w3, scalar=NEG_SLOPE, in1=w3, op0=mult, op1=amax)
            nc.scalar.activation(out=wst[:, 0:Tg * 8], in_=wst[:, 0:Tg * 8],
                                 func=AF.Exp)

            ust = sbg.tile([128, TGMAX * U1], F16, tag="ust", name="ust")
            nc.vector.tensor_tensor(
                out=ust[:, 0:Tg * U1].rearrange("p (t e) -> p t e", e=U1)
                [:, :, 0:128].rearrange("p t (h c) -> p t h c", c=NHID),
                in0=g13[:, :, 0:128].rearrange("p t (h c) -> p t h c", c=NHID),
                in1=wst[:, 0:Tg * 8].rearrange("p (t h c) -> p t h c", h=8, c=1)
                .to_broadcast([128, Tg, 8, NHID]),
                op=mult)
            nc.vector.tensor_copy(
                ust[:, 0:Tg * U1].rearrange("p (t e) -> p t e", e=U1)
                [:, :, 128:136], w3)

            for c4 in range(G1):
                d = g * G1 + c4
                if d >= NCH:
                    break
                segs, indlen = _chunk_tiles(ranges, shapes2, g, c4)
                if not segs:
                    nc.vector.tensor_copy(h1acc[:, d * U1:(d + 1) * U1],
                                          selfacc[:, d * U1:(d + 1) * U1])
                    continue
                ind = sbg.tile([128, TCMAX * 128], F16, tag="ind", name="ind")
                for (s, gloc, ioff, ntl) in segs:
                    nc.vector.tensor_tensor(
                        out=ind[:, ioff * 128:(ioff + ntl) * 128]
                        .rearrange("p (t s) -> p t s", s=128),
                        in0=dlc[:, gloc:gloc + ntl]
                        .rearrange("p (t s) -> p t s", s=1)
                        .to_broadcast([128, ntl, 128]),
                        in1=iotaa[:, d * 128:(d + 1) * 128]
                        .rearrange("p (t s) -> p t s", t=1)
                        .to_broadcast([128, ntl, 128]),
                        op=eq)
                ps1 = psc.tile([128, U1], F32, tag="ps1", name="ps1")
                j = 0
                ntot = sum(x[3] for x in segs)
                for (s, gloc, ioff, ntl) in segs:
                    for t in range(ntl):
                        nc.tensor.matmul(
                            ps1[:],
                            lhsT=ind[:, (ioff + t) * 128:(ioff + t + 1) * 128],
                            rhs=ust[:, (gloc + t) * U1:(gloc + t + 1) * U1],
                            start=(j == 0), stop=(j == ntot - 1))
                        j += 1
                nc.vector.tensor_tensor(
                    out=h1acc[:, d * U1:(d + 1) * U1], in0=ps1[:],
                    in1=selfacc[:, d * U1:(d + 1) * U1], op=aadd)

    with tc.tile_pool(name="sbC2", bufs=1) as sb2, \
         tc.tile_pool(name="psC2", bufs=2, space="PSUM") as ps2p:
        h3 = h1acc[:].rearrange("p (d e) -> p d e", e=U1)
        rc = sb2.tile([128, NCH * 8], F16, tag="rc", name="rc")
        rc3 = rc[:].rearrange("p (d h) -> p d h", h=8)
        with nc.allow_low_precision(reason="fp16 recip of O(100) softmax sums"):
            nc.vector.reciprocal(rc3, h3[:, :, 128:136])
        o1 = sb2.tile([128, NCH * 128], F16, tag="o1", name="o1")
        nc.vector.tensor_tensor(
            out=o1[:].rearrange("p (d h c) -> p d h c", h=8, c=NHID),
            in0=h3[:, :, 0:128].rearrange("p d (h c) -> p d h c", c=NHID),
            in1=rc[:].rearrange("p (d h c) -> p d h c", h=8, c=1)
            .to_broadcast([128, NCH, 8, NHID]),
            op=mult)
        o3 = o1[:].rearrange("p (d e) -> p d e", e=128)
        nc.vector.tensor_tensor(
            out=o3, in0=o3,
            in1=b1t[:].rearrange("p (d e) -> p d e", d=1)
            .to_broadcast([128, NCH, 128]),
            op=aadd)
        t2 = sb2.tile([128, NCH * 128], F16, tag="t2", name="t2")
        nc.vector.tensor_scalar_min(t2[:], o1[:], 0.0)
        nc.scalar.activation(out=t2[:], in_=t2[:], func=AF.Exp)
        nc.vector.tensor_scalar_add(t2[:], t2[:], -1.0)
        nc.vector.tensor_scalar_max(o1[:], o1[:], 0.0)
        nc.vector.tensor_tensor(out=o1[:], in0=o1[:], in1=t2[:], op=aadd)

        from concourse.masks import make_identity
        identt = sb2.tile([128, 128], F16, tag="id2", name="id2")
        make_identity(nc, identt[:])
        for d in range(NCH):
            psT = ps2p.tile([128, 128], F16, tag="psT", name="psT")
            nc.tensor.transpose(psT[:], o1[:, d * 128:(d + 1) * 128], identt[:])
            eluT = sb2.tile([128, 128], F16, tag="eluT", name="eluT")
            nc.scalar.activation(out=eluT[:], in_=psT[:], func=AF.Copy)
            ps2a = ps2p.tile([128, 43], F32, tag="ps2a", name="ps2a")
            nc.tensor.matmul(ps2a[:], lhsT=eluT[:], rhs=w2s[:],
                             start=True, stop=True)
            nc.scalar.activation(out=h2acc[:, d * 43:(d + 1) * 43],
                                 in_=ps2a[:], func=AF.Copy)
        hh3 = h2acc[:].rearrange("p (d e) -> p d e", e=43)
        sw2 = sb2.tile([128, NCH], F16, tag="sw2", name="sw2")
        sw23 = sw2[:].rearrange("p (d h) -> p d h", h=1)
        nc.vector.tensor_tensor(out=sw23, in0=hh3[:, :, 41:42],
                                in1=hh3[:, :, 42:43], op=aadd)
        nc.vector.scalar_tensor_tensor(
            out=sw23, in0=sw23, scalar=NEG_SLOPE, in1=sw23,
            op0=mult, op1=amax)
        nc.scalar.activation(out=sw2[:], in_=sw2[:], func=AF.Exp)
        s23 = self2acc[:].rearrange("p (d e) -> p d e", e=NCLASS + 1)
        nc.vector.tensor_tensor(
            out=s23[:, :, 0:NCLASS], in0=hh3[:, :, 0:NCLASS],
            in1=sw23.to_broadcast([128, NCH, NCLASS]), op=mult)
        nc.vector.tensor_copy(s23[:, :, NCLASS:NCLASS + 1], sw23)
        nc.vector.memset(hh3[:, :, 40:41], 1.0)
        nc.sync.dma_start(
            tab2_sh.rearrange("(d p) e -> p d e", p=128),
            hh3[:, :, 0:U2])
        nc.sync.dma_start(
            adr2.rearrange("(d p) e -> p d e", p=128)[:, :, 0:1],
            hh3[:, :, 42:43])


def _phase_e(nc, tc, shapes2, maxcnt2, ranges, g_tiles, g_off, TGMAX, TCMAX,
             IDX1, IDXD, DSTI, iotaa, b2t, tab2, adr2, oacc, OUT, self2acc,
             eq, mult, amax, aadd, sub, AF, AX):
    ng = shapes2.shape[0]
    NC1 = NCLASS + 1
    with tc.tile_pool(name="sbE", bufs=2) as sbg, \
         tc.tile_pool(name="psE", bufs=4, space="PSUM") as pse:
        for g in range(ng):
            Tg = g_tiles[g]
            if Tg == 0:
                continue
            goff = g_off[g]
            i1 = sbg.tile([128, TGMAX * 8], I16, tag="i1e", name="i1e")
            nc.sync.dma_start(i1[:, 0:Tg * 8], IDX1[:, goff * 8:(goff + Tg) * 8])
            idd = sbg.tile([128, TGMAX * 8], I16, tag="idde", name="idde")
            nc.sync.dma_start(idd[:, 0:Tg * 8], IDXD[:, goff * 8:(goff + Tg) * 8])
            dlc = sbg.tile([128, TGMAX], I16, tag="dlce", name="dlce")
            nc.sync.dma_start(dlc[:, 0:Tg], DSTI[:, goff:goff + Tg])

            g2 = sbg.tile([128, TGMAX * U2], F16, tag="g2", name="g2")
            ga2 = sbg.tile([128, TGMAX * 8], F16, tag="ga2", name="ga2")
            if g < 2:
                nc.vector.memset(g2[:], 0.0)
                nc.vector.memset(ga2[:], 0.0)
            loc = 0
            for s in range(NSCH):
                cl = int(shapes2[g, s])
                if cl == 0:
                    continue
                mx = int(maxcnt2[g, s])
                n16 = -(-mx // 16) * 16
                nt = -(-n16 // 128)
                _dma_gather_raw(
                    nc.gpsimd,
                    g2[:, (loc // 128) * U2:((loc // 128) + nt) * U2]
                    .rearrange("p (t e) -> p t e", e=U2),
                    tab2[s * SCHW:(s + 1) * SCHW, :],
                    i1[:, loc // 16:(loc + n16) // 16], n16, mx, U2, ROW2,
                    queue_num=s)
                _dma_gather_raw(
                    nc.gpsimd,
                    ga2[:, (loc // 128) * 8:((loc // 128) + nt) * 8]
                    .rearrange("p (t e) -> p t e", e=8),
                    adr2[:],
                    idd[:, loc // 16:(loc + n16) // 16], n16, mx, 8, ROWA,
                    queue_num=(s + 2) % 4)
                loc += cl

            g23 = g2[:, 0:Tg * U2].rearrange("p (t e) -> p t e", e=U2)
            ga23 = ga2[:, 0:Tg * 8].rearrange("p (t e) -> p t e", e=8)

            w2t = sbg.tile([128, TGMAX], F16, tag="w2t", name="w2t")
            wt3 = w2t[:, 0:Tg].rearrange("p (t h) -> p t h", h=1)
            nc.vector.tensor_tensor(out=wt3, in0=g23[:, :, 41:42],
                                    in1=ga23[:, :, 0:1], op=aadd)
            nc.vector.scalar_tensor_tensor(
                out=wt3, in0=wt3, scalar=NEG_SLOPE, in1=wt3,
                op0=mult, op1=amax)
            nc.scalar.activation(out=w2t[:, 0:Tg], in_=w2t[:, 0:Tg],
                                 func=AF.Exp)

            gw = sbg.tile([128, TGMAX * NC1], F16, tag="gw", name="gw")
            nc.vector.tensor_tensor(
                out=gw[:, 0:Tg * NC1].rearrange("p (t e) -> p t e", e=NC1),
                in0=g23[:, :, 0:NC1],
                in1=w2t[:, 0:Tg].rearrange("p (t s) -> p t s", s=1)
                .to_broadcast([128, Tg, NC1]),
                op=mult)

            for c4 in range(G1):
                d = g * G1 + c4
                if d >= NCH:
                    break
                segs, indlen = _chunk_tiles(ranges, shapes2, g, c4)
                if not segs:
                    nc.vector.tensor_copy(oacc[:, d * NC1:(d + 1) * NC1],
                                          self2acc[:, d * NC1:(d + 1) * NC1])
                    continue
                ind = sbg.tile([128, TCMAX * 128], F16, tag="inde", name="inde")
                for (s, gloc, ioff, ntl) in segs:
                    nc.vector.tensor_tensor(
                        out=ind[:, ioff * 128:(ioff + ntl) * 128]
                        .rearrange("p (t s) -> p t s", s=128),
                        in0=dlc[:, gloc:gloc + ntl]
                        .rearrange("p (t s) -> p t s", s=1)
                        .to_broadcast([128, ntl, 128]),
                        in1=iotaa[:, d * 128:(d + 1) * 128]
                        .rearrange("p (t s) -> p t s", t=1)
                        .to_broadcast([128, ntl, 128]),
                        op=eq)
                ps2 = pse.tile([128, NC1], F32, tag="ps2", name="ps2")
                j = 0
                ntot = sum(x[3] for x in segs)
                for (s, gloc, ioff, ntl) in segs:
                    for t in range(ntl):
                        nc.tensor.matmul(
                            ps2[:],
                            lhsT=ind[:, (ioff + t) * 128:(ioff + t + 1) * 128],
                            rhs=gw[:, (gloc + t) * NC1:(gloc + t + 1) * NC1],
                            start=(j == 0), stop=(j == ntot - 1))
                        j += 1
                nc.vector.tensor_tensor(
                    out=oacc[:, d * NC1:(d + 1) * NC1], in0=ps2[:],
                    in1=self2acc[:, d * NC1:(d + 1) * NC1], op=aadd)

    with tc.tile_pool(name="sbE2", bufs=1) as sb2:
        oa3 = oacc[:].rearrange("p (d e) -> p d e", e=NC1)
        rc2 = sb2.tile([128, NCH], F16, tag="rc2", name="rc2")
        with nc.allow_low_precision(reason="fp16 recip of O(100) softmax sums"):
            nc.vector.reciprocal(rc2[:].rearrange("p (d h) -> p d h", h=1),
                                 oa3[:, :, NCLASS:NC1])
        lg = sb2.tile([128, NCH * NCLASS], F32, tag="lg", name="lg")
        lg3 = lg[:].rearrange("p (d e) -> p d e", e=NCLASS)
        nc.vector.tensor_tensor(
            out=lg3, in0=oa3[:, :, 0:NCLASS],
            in1=rc2[:].rearrange("p (d h) -> p d h", h=1)
            .to_broadcast([128, NCH, NCLASS]),
            op=mult)
        nc.vector.tensor_tensor(
            out=lg3, in0=lg3,
            in1=b2t[:].rearrange("p (d e) -> p d e", d=1)
            .to_broadcast([128, NCH, NCLASS]),
            op=aadd)
        ex = sb2.tile([128, NCH * NCLASS], F32, tag="ex", name="ex")
        nc.scalar.activation(out=ex[:], in_=lg[:], func=AF.Exp)
        sm = sb2.tile([128, NCH], F32, tag="sm", name="sm")
        nc.vector.tensor_reduce(
            out=sm[:], in_=ex[:].rearrange("p (d e) -> p d e", e=NCLASS),
            axis=AX.X, op=aadd)
        ln = sb2.tile([128, NCH], F32, tag="ln", name="ln")
        nc.scalar.activation(out=ln[:], in_=sm[:], func=AF.Ln)
        nc.vector.tensor_tensor(
            out=lg3, in0=lg3,
            in1=ln[:].rearrange("p (d h) -> p d h", h=1)
            .to_broadcast([128, NCH, NCLASS]),
            op=sub)
        nc.sync.dma_start(
            OUT.rearrange("(d p) e -> p d e", p=128), lg3)


_CACHE = {}


def kernel(x, edge_index, W1, att_src1, att_dst1, b1, W2, att_src2, att_dst2, b2):
    x = np.asarray(x, dtype=np.float32)
    edge_index = np.asarray(edge_index)
    in_maps, meta = _prep(np.asarray(x), edge_index,
                          np.asarray(W1), np.asarray(att_src1),
                          np.asarray(att_dst1), np.asarray(W2),
                          np.asarray(att_src2), np.asarray(att_dst2),
                          b1=b1, b2=b2)
    key = (meta[0].tobytes(), meta[1].tobytes(), meta[2].tobytes())
    if key not in _CACHE:
        _CACHE[key] = _build(meta)
    nc = _CACHE[key]
    res = run_bass_kernel_spmd(nc, in_maps, core_ids=list(range(NCORES)))
    out = np.concatenate([res.results[k]["out"][:NPC] for k in range(NCORES)],
                         axis=0)
    return out.astype(np.float32)
